# revision 5
# baseline (speedup 1.0000x reference)
"""Trainium2 Bass kernel for nn_Brick_Wall_Network.

Math: the reference builds a 16x16 complex unitary U from 12 scalars,
computes a = Re(U @ x) (x real => a = Re(U) @ x), and returns
a.T @ (M @ a) with M = I8 (x) Z = diag(+1,-1,...).  Folding:
    out = x.T @ G @ x,   G = Ur.T @ M @ Ur   (16x16 symmetric, Ur = Re(U))
The G build is O(16x16) scalar work replicated on host; all O(B^2)
work runs on the 8 NeuronCores.

Sharding: pure data parallelism over output rows.  Core i computes
out[i*1024:(i+1)*1024, :] = (G @ x[:, rows_i]).T @ x  via the tensor
engine; host concatenates the 8 row blocks.

All per-core inputs are packed into ONE DRAM tensor (one DMA, one
completion semaphore): walrus can encode only a single sync-wait on a
Matmult, so matmuls may depend on at most one semaphore.
"""

import numpy as np

_NCORES = 8
_B = 8192
_SH = _B // _NCORES  # 1024 output rows per core
# packed input layout: [ G (16) | xr (_SH) | x (_B) ] along the free dim
_OFF_G = 0
_OFF_XR = 16
_OFF_X = 16 + _SH
_W = 16 + _SH + _B
_CACHE = {}


def _build_G(phi_1, theta_1, omega_1, phi_2, theta_2, omega_2,
             phi_3, theta_3, omega_3, phi_4, theta_4, omega_4):
    def u(phi, theta, omega):
        phi = float(np.asarray(phi).reshape(-1)[0])
        theta = float(np.asarray(theta).reshape(-1)[0])
        omega = float(np.asarray(omega).reshape(-1)[0])
        half = theta / 2.0
        c, s = np.cos(half), np.sin(half)
        return np.array(
            [[c * np.exp(-1j * (phi + omega) / 2), -s * np.exp(1j * (phi - omega) / 2)],
             [s * np.exp(-1j * (phi - omega) / 2), c * np.exp(1j * (phi + omega) / 2)]],
            dtype=np.complex128)

    CNOT = np.array([[1, 0, 0, 0], [0, 1, 0, 0], [0, 0, 0, 1], [0, 0, 1, 0]],
                    dtype=np.float64)
    I2 = np.eye(2, dtype=np.float64)
    Z = np.array([[1.0, 0.0], [0.0, -1.0]], dtype=np.float64)

    g1 = u(phi_1, theta_1, omega_1)
    # NOTE: gate 2 intentionally uses (phi_2, theta_3, omega_4), as in the source.
    g2 = u(phi_2, theta_3, omega_4)
    g3 = u(phi_3, theta_3, omega_3)
    g4 = u(phi_4, theta_4, omega_4)
    layer_1 = np.kron(np.kron(np.kron(g1, g2), g3), g4)

    layer_2 = np.kron(np.kron(CNOT, I2), I2).astype(np.complex128)
    layer_3 = np.kron(np.kron(I2, CNOT), I2).astype(np.complex128)
    l4_real = np.kron(np.kron(I2, I2), CNOT)
    layer_4 = l4_real.astype(np.complex128)
    layer_5 = (l4_real.reshape((2,) * 8).transpose(0, 2, 1, 3, 4, 6, 5, 7)
               .reshape(16, 16).astype(np.complex128))

    U = layer_5 @ (layer_4 @ (layer_3 @ (layer_2 @ layer_1)))
    M = np.kron(np.kron(np.kron(I2, I2), I2), Z)
    Ur = np.real(U)
    G = Ur.T @ (M @ Ur)
    return np.ascontiguousarray(G, dtype=np.float32)


def _legalize_waits(nc):
    """This walrus build encodes at most ONE sync-wait per instruction
    (setupSyncWait raises "Too many sync wait commands" for 2+), while the
    Tile scheduler freely attaches several.  Hoist all-but-one wait of every
    multi-wait instruction onto same-engine NoOps placed right before it —
    sequencers execute in order, so semantics are unchanged."""
    import concourse.mybir as mybir

    n = 0
    for fn in nc.m.functions:
        for b in fn.blocks:
            new = []
            for inst in b.instructions:
                si = getattr(inst, "sync_info", None)
                ow = list(si.on_wait) if (si is not None and si.on_wait) else []
                if len(ow) > 1:
                    for w in ow[:-1]:
                        nop = mybir.InstNoOp()
                        nop.engine = inst.engine
                        nop.name = f"legal-nop-{n}"
                        nop.sync_info = mybir.SyncInfo(on_wait=[w], on_update=[])
                        new.append(nop)
                        n += 1
                    si.on_wait = [ow[-1]]
                new.append(inst)
            b.instructions[:] = new
    return nc


def _get_nc():
    if "nc" in _CACHE:
        return _CACHE["nc"]
    import concourse.bass as bass
    import concourse.tile as tile
    import concourse.mybir as mybir

    f32 = mybir.dt.float32
    nc = bass.Bass()
    w_d = nc.dram_tensor("w", [16, _W], f32, kind="ExternalInput")
    out_d = nc.dram_tensor("out", [_SH, _B], f32, kind="ExternalOutput")

    with tile.TileContext(nc) as tc:
        with tc.tile_pool(name="const", bufs=1) as cpool, \
             tc.tile_pool(name="outp", bufs=2) as opool, \
             tc.tile_pool(name="ps_small", bufs=2, space="PSUM") as pss, \
             tc.tile_pool(name="ps_big", bufs=6, space="PSUM") as psb:
            w_sb = cpool.tile([16, _W], f32)
            nc.sync.dma_start(out=w_sb[:], in_=w_d[:])
            g_sb = w_sb[:, _OFF_G:_OFF_G + 16]
            xr_sb = w_sb[:, _OFF_XR:_OFF_XR + _SH]
            x_sb = w_sb[:, _OFF_X:_OFF_X + _B]

            # br = G @ xr  (16, _SH); stationary for the main matmuls.
            br_sb = cpool.tile([16, _SH], f32)
            for j in range(_SH // 512):
                ps = pss.tile([16, 512], f32)
                nc.tensor.matmul(ps[:], g_sb, xr_sb[:, j * 512:(j + 1) * 512],
                                 start=True, stop=True)
                nc.vector.tensor_copy(br_sb[:, j * 512:(j + 1) * 512], ps[:])

            # out rows block m: (128, B) = br[:, m*128:...].T @ x
            for m in range(_SH // 128):
                o_sb = opool.tile([128, _B], f32)
                for n in range(_B // 512):
                    ps = psb.tile([128, 512], f32)
                    nc.tensor.matmul(ps[:], br_sb[:, m * 128:(m + 1) * 128],
                                     x_sb[:, n * 512:(n + 1) * 512],
                                     start=True, stop=True)
                    if n % 2 == 0:
                        nc.vector.tensor_copy(o_sb[:, n * 512:(n + 1) * 512], ps[:])
                    else:
                        nc.scalar.copy(o_sb[:, n * 512:(n + 1) * 512], ps[:])
                nc.sync.dma_start(out=out_d[m * 128:(m + 1) * 128, :], in_=o_sb[:])

    _legalize_waits(nc)
    _CACHE["nc"] = nc
    return nc


def _run(inputs, trace=False, **kw):
    from concourse.bass_utils import run_bass_kernel_spmd

    x = np.ascontiguousarray(np.asarray(inputs["input_state"], dtype=np.float32))
    G = _build_G(**{k: v for k, v in inputs.items() if k != "input_state"})
    nc = _get_nc()
    in_maps = []
    for i in range(_NCORES):
        w = np.empty((16, _W), dtype=np.float32)
        w[:, _OFF_G:_OFF_G + 16] = G
        w[:, _OFF_XR:_OFF_XR + _SH] = x[:, i * _SH:(i + 1) * _SH]
        w[:, _OFF_X:_OFF_X + _B] = x
        in_maps.append({"w": w})
    res = run_bass_kernel_spmd(nc, in_maps, core_ids=list(range(_NCORES)),
                               trace=trace, **kw)
    out = np.concatenate([res.results[i]["out"] for i in range(_NCORES)], axis=0)
    return np.ascontiguousarray(out, dtype=np.float32), res


# ---------------------------------------------------------------------------
# v2: exploit output symmetry (out = x.T G x with G symmetric => out = out.T).
# Only the 136 upper-triangle 512x512 tiles are computed on device (~halving
# the dominant HBM write traffic); the host mirrors the strict-lower tiles.
# Core i owns row-blocks {i, 15-i}: (16-i) + (i+1) = 17 tiles on every core,
# so the SPMD program is identical across cores; which (r, c) tile slot t
# holds is pure input data (per-tile x-column slices packed by the host).
# ---------------------------------------------------------------------------

_NT = 17  # tiles per core
_TS = 512  # tile side
# packed layout per partition row: [ G (16) | xrows (17*512) | xcols (17*512) ]
_OFF2_G = 0
_OFF2_XR = 16
_OFF2_XC = 16 + _NT * _TS
_W2 = 16 + 2 * _NT * _TS


def _tiles_for_core(i):
    r1, r2 = i, 15 - i
    return ([(r1, c) for c in range(r1, 16)] +
            [(r2, c) for c in range(r2, 16)])


def _get_nc2():
    if "nc2" in _CACHE:
        return _CACHE["nc2"]
    import concourse.bass as bass
    import concourse.tile as tile
    import concourse.mybir as mybir

    f32 = mybir.dt.float32
    nc = bass.Bass()
    w_d = nc.dram_tensor("w", [16, _W2], f32, kind="ExternalInput")
    # p-major tile store: out[p, t, m*512+j] = tile_t[m*128+p, j]
    out_d = nc.dram_tensor("out", [128, _NT, 4 * _TS], f32, kind="ExternalOutput")

    with tile.TileContext(nc) as tc:
        with tc.tile_pool(name="const", bufs=1) as cpool, \
             tc.tile_pool(name="bt", bufs=2) as btpool, \
             tc.tile_pool(name="outp", bufs=3) as opool, \
             tc.tile_pool(name="ps_small", bufs=2, space="PSUM") as pss, \
             tc.tile_pool(name="ps_big", bufs=6, space="PSUM") as psb:
            w_sb = cpool.tile([16, _W2], f32)
            nc.sync.dma_start(out=w_sb[:], in_=w_d[:])
            g_sb = w_sb[:, _OFF2_G:_OFF2_G + 16]

            for t in range(_NT):
                xrow_t = w_sb[:, _OFF2_XR + t * _TS:_OFF2_XR + (t + 1) * _TS]
                xcol_t = w_sb[:, _OFF2_XC + t * _TS:_OFF2_XC + (t + 1) * _TS]
                # bt = G @ xrows[t]  (16, 512)
                ps_s = pss.tile([16, _TS], f32)
                nc.tensor.matmul(ps_s[:], g_sb, xrow_t, start=True, stop=True)
                bt_sb = btpool.tile([16, _TS], f32)
                nc.vector.tensor_copy(bt_sb[:], ps_s[:])
                # tile(r,c) = bt.T @ xcols[t]  (512, 512), in 4 psum chunks
                o_sb = opool.tile([128, 4 * _TS], f32)
                for m in range(4):
                    ps = psb.tile([128, _TS], f32)
                    nc.tensor.matmul(ps[:], bt_sb[:, m * 128:(m + 1) * 128],
                                     xcol_t, start=True, stop=True)
                    nc.vector.tensor_copy(o_sb[:, m * _TS:(m + 1) * _TS], ps[:])
                nc.sync.dma_start(out=out_d[:, t, :], in_=o_sb[:])

    _legalize_waits(nc)
    _CACHE["nc2"] = nc
    return nc


def _run2(inputs, trace=False, **kw):
    from concourse.bass_utils import run_bass_kernel_spmd

    x = np.ascontiguousarray(np.asarray(inputs["input_state"], dtype=np.float32))
    G = _build_G(**{k: v for k, v in inputs.items() if k != "input_state"})
    nc = _get_nc2()
    in_maps = []
    for i in range(_NCORES):
        w = np.empty((16, _W2), dtype=np.float32)
        w[:, _OFF2_G:_OFF2_G + 16] = G
        for t, (r, c) in enumerate(_tiles_for_core(i)):
            w[:, _OFF2_XR + t * _TS:_OFF2_XR + (t + 1) * _TS] = \
                x[:, r * _TS:(r + 1) * _TS]
            w[:, _OFF2_XC + t * _TS:_OFF2_XC + (t + 1) * _TS] = \
                x[:, c * _TS:(c + 1) * _TS]
        in_maps.append({"w": w})
    res = run_bass_kernel_spmd(nc, in_maps, core_ids=list(range(_NCORES)),
                               trace=trace, **kw)
    out = np.empty((_B, _B), dtype=np.float32)
    for i in range(_NCORES):
        arr = res.results[i]["out"]  # (128, 17, 2048)
        tiles = np.ascontiguousarray(
            arr.reshape(128, _NT, 4, _TS).transpose(1, 2, 0, 3)
        ).reshape(_NT, _TS, _TS)
        for t, (r, c) in enumerate(_tiles_for_core(i)):
            out[r * _TS:(r + 1) * _TS, c * _TS:(c + 1) * _TS] = tiles[t]
            if c > r:
                out[c * _TS:(c + 1) * _TS, r * _TS:(r + 1) * _TS] = tiles[t].T
    return out, res


_VARIANTS = {"v1": _run, "v2": _run2}


def kernel(**inputs):
    import os
    run = _VARIANTS[os.environ.get("BWN_VARIANT", "v1")]
    out, _ = run(inputs)
    return out


# revision 8
# speedup vs baseline: 1.8194x; 1.8194x over previous
"""Trainium2 Bass kernel for nn_Brick_Wall_Network.

Math: the reference builds a 16x16 complex unitary U from 12 scalars,
computes a = Re(U @ x) (x real => a = Re(U) @ x), and returns
a.T @ (M @ a) with M = I8 (x) Z = diag(+1,-1,...).  Folding:
    out = x.T @ G @ x,   G = Ur.T @ M @ Ur   (16x16 symmetric, Ur = Re(U))
The G build is O(16x16) scalar work replicated on host; all O(B^2)
work runs on the 8 NeuronCores.

v1: row-sharded full output, fp32 matmuls (accuracy-gold baseline).
v2 (default): output is symmetric, so only the 136 upper-triangle
    512x512 tiles are computed (17 per core — perfectly balanced),
    host mirrors the rest.  Matmuls run as bf16 hi/lo 3-pass
    (hi*hi + hi*lo + lo*hi), giving ~1e-5 relative accuracy at the
    bf16 streaming rate (fp32 matmul is ~4x slower per column).

This walrus build encodes at most ONE sync-wait per instruction, while
the Tile scheduler attaches several; _legalize_waits() hoists extras
onto same-engine NoOps.
"""

import numpy as np

_NCORES = 8
_B = 8192
_SH = _B // _NCORES  # 1024 output rows per core (v1)
_CACHE = {}

# v1 packed input layout: [ G (16) | xr (_SH) | x (_B) ] along the free dim
_OFF_G = 0
_OFF_XR = 16
_OFF_X = 16 + _SH
_W = 16 + _SH + _B

# v2: 16 row-blocks of 512; core i owns row-blocks {i, 15-i} => 17 tiles/core
_NT = 17
_TS = 512
# v2 packed bf16 layout: [Gh(16) | Gl(16) | xrh | xrl | xch | xcl]
_L = _NT * _TS
_OFF2_GH = 0
_OFF2_GL = 16
_OFF2_XRH = 32
_OFF2_XRL = 32 + _L
_OFF2_XCH = 32 + 2 * _L
_OFF2_XCL = 32 + 3 * _L
_W2 = 32 + 4 * _L


def _build_G(phi_1, theta_1, omega_1, phi_2, theta_2, omega_2,
             phi_3, theta_3, omega_3, phi_4, theta_4, omega_4):
    def u(phi, theta, omega):
        phi = float(np.asarray(phi).reshape(-1)[0])
        theta = float(np.asarray(theta).reshape(-1)[0])
        omega = float(np.asarray(omega).reshape(-1)[0])
        half = theta / 2.0
        c, s = np.cos(half), np.sin(half)
        return np.array(
            [[c * np.exp(-1j * (phi + omega) / 2), -s * np.exp(1j * (phi - omega) / 2)],
             [s * np.exp(-1j * (phi - omega) / 2), c * np.exp(1j * (phi + omega) / 2)]],
            dtype=np.complex128)

    CNOT = np.array([[1, 0, 0, 0], [0, 1, 0, 0], [0, 0, 0, 1], [0, 0, 1, 0]],
                    dtype=np.float64)
    I2 = np.eye(2, dtype=np.float64)
    Z = np.array([[1.0, 0.0], [0.0, -1.0]], dtype=np.float64)

    g1 = u(phi_1, theta_1, omega_1)
    # NOTE: gate 2 intentionally uses (phi_2, theta_3, omega_4), as in the source.
    g2 = u(phi_2, theta_3, omega_4)
    g3 = u(phi_3, theta_3, omega_3)
    g4 = u(phi_4, theta_4, omega_4)
    layer_1 = np.kron(np.kron(np.kron(g1, g2), g3), g4)

    layer_2 = np.kron(np.kron(CNOT, I2), I2).astype(np.complex128)
    layer_3 = np.kron(np.kron(I2, CNOT), I2).astype(np.complex128)
    l4_real = np.kron(np.kron(I2, I2), CNOT)
    layer_4 = l4_real.astype(np.complex128)
    layer_5 = (l4_real.reshape((2,) * 8).transpose(0, 2, 1, 3, 4, 6, 5, 7)
               .reshape(16, 16).astype(np.complex128))

    U = layer_5 @ (layer_4 @ (layer_3 @ (layer_2 @ layer_1)))
    M = np.kron(np.kron(np.kron(I2, I2), I2), Z)
    Ur = np.real(U)
    G = Ur.T @ (M @ Ur)
    return np.ascontiguousarray(G, dtype=np.float32)


def _legalize_waits(nc):
    """walrus here encodes at most ONE sync-wait per instruction; hoist the
    extras onto same-engine NoOps placed just before the instruction."""
    import concourse.mybir as mybir

    n = 0
    for fn in nc.m.functions:
        for b in fn.blocks:
            new = []
            for inst in b.instructions:
                si = getattr(inst, "sync_info", None)
                ow = list(si.on_wait) if (si is not None and si.on_wait) else []
                if len(ow) > 1:
                    for w in ow[:-1]:
                        nop = mybir.InstNoOp()
                        nop.engine = inst.engine
                        nop.name = f"legal-nop-{n}"
                        nop.sync_info = mybir.SyncInfo(on_wait=[w], on_update=[])
                        new.append(nop)
                        n += 1
                    si.on_wait = [ow[-1]]
                new.append(inst)
            b.instructions[:] = new
    return nc


# ---------------------------------------------------------------------------
# v1: full output, fp32 matmuls
# ---------------------------------------------------------------------------

def _get_nc():
    if "nc" in _CACHE:
        return _CACHE["nc"]
    import concourse.bass as bass
    import concourse.tile as tile
    import concourse.mybir as mybir

    f32 = mybir.dt.float32
    nc = bass.Bass()
    w_d = nc.dram_tensor("w", [16, _W], f32, kind="ExternalInput")
    out_d = nc.dram_tensor("out", [_SH, _B], f32, kind="ExternalOutput")

    with tile.TileContext(nc) as tc:
        with tc.tile_pool(name="const", bufs=1) as cpool, \
             tc.tile_pool(name="outp", bufs=2) as opool, \
             tc.tile_pool(name="ps_small", bufs=2, space="PSUM") as pss, \
             tc.tile_pool(name="ps_big", bufs=6, space="PSUM") as psb:
            w_sb = cpool.tile([16, _W], f32)
            nc.sync.dma_start(out=w_sb[:], in_=w_d[:])
            g_sb = w_sb[:, _OFF_G:_OFF_G + 16]
            xr_sb = w_sb[:, _OFF_XR:_OFF_XR + _SH]
            x_sb = w_sb[:, _OFF_X:_OFF_X + _B]

            # br = G @ xr  (16, _SH); stationary for the main matmuls.
            br_sb = cpool.tile([16, _SH], f32)
            for j in range(_SH // 512):
                ps = pss.tile([16, 512], f32)
                nc.tensor.matmul(ps[:], g_sb, xr_sb[:, j * 512:(j + 1) * 512],
                                 start=True, stop=True)
                nc.vector.tensor_copy(br_sb[:, j * 512:(j + 1) * 512], ps[:])

            # out rows block m: (128, B) = br[:, m*128:...].T @ x
            for m in range(_SH // 128):
                o_sb = opool.tile([128, _B], f32)
                for n in range(_B // 512):
                    ps = psb.tile([128, 512], f32)
                    nc.tensor.matmul(ps[:], br_sb[:, m * 128:(m + 1) * 128],
                                     x_sb[:, n * 512:(n + 1) * 512],
                                     start=True, stop=True)
                    if n % 2 == 0:
                        nc.vector.tensor_copy(o_sb[:, n * 512:(n + 1) * 512], ps[:])
                    else:
                        nc.scalar.copy(o_sb[:, n * 512:(n + 1) * 512], ps[:])
                nc.sync.dma_start(out=out_d[m * 128:(m + 1) * 128, :], in_=o_sb[:])

    _legalize_waits(nc)
    _CACHE["nc"] = nc
    return nc


def _run(inputs, trace=False, **kw):
    from concourse.bass_utils import run_bass_kernel_spmd

    x = np.ascontiguousarray(np.asarray(inputs["input_state"], dtype=np.float32))
    G = _build_G(**{k: v for k, v in inputs.items() if k != "input_state"})
    nc = _get_nc()
    in_maps = []
    for i in range(_NCORES):
        w = np.empty((16, _W), dtype=np.float32)
        w[:, _OFF_G:_OFF_G + 16] = G
        w[:, _OFF_XR:_OFF_XR + _SH] = x[:, i * _SH:(i + 1) * _SH]
        w[:, _OFF_X:_OFF_X + _B] = x
        in_maps.append({"w": w})
    res = run_bass_kernel_spmd(nc, in_maps, core_ids=list(range(_NCORES)),
                               trace=trace, **kw)
    out = np.concatenate([res.results[i]["out"] for i in range(_NCORES)], axis=0)
    return np.ascontiguousarray(out, dtype=np.float32), res


# ---------------------------------------------------------------------------
# v2: symmetric upper-triangle tiles + bf16 hi/lo 3-pass matmuls
# ---------------------------------------------------------------------------

def _tiles_for_core(i):
    r1, r2 = i, 15 - i
    return ([(r1, c) for c in range(r1, 16)] +
            [(r2, c) for c in range(r2, 16)])


def _get_nc2():
    if "nc2" in _CACHE:
        return _CACHE["nc2"]
    import concourse.bass as bass
    import concourse.tile as tile
    import concourse.mybir as mybir

    f32 = mybir.dt.float32
    bf16 = mybir.dt.bfloat16
    sub = mybir.AluOpType.subtract
    nc = bass.Bass()
    w_d = nc.dram_tensor("w", [16, _W2], bf16, kind="ExternalInput")
    # p-major tile store: out[p, t, m*512+j] = tile_t[m*128+p, j]
    out_d = nc.dram_tensor("out", [128, _NT, 4 * _TS], f32, kind="ExternalOutput")

    with tile.TileContext(nc) as tc:
        with tc.tile_pool(name="const", bufs=1) as cpool, \
             tc.tile_pool(name="bt", bufs=3) as btpool, \
             tc.tile_pool(name="outp", bufs=3) as opool, \
             tc.tile_pool(name="outp1", bufs=1) as opool1, \
             tc.tile_pool(name="ps_small", bufs=2, space="PSUM") as pss, \
             tc.tile_pool(name="ps_big", bufs=6, space="PSUM") as psb:
            w_sb = cpool.tile([16, _W2], bf16)
            nc.sync.dma_start(out=w_sb[:], in_=w_d[:])
            gh = w_sb[:, _OFF2_GH:_OFF2_GH + 16]
            gl = w_sb[:, _OFF2_GL:_OFF2_GL + 16]

            def do_tile(t, o_sb, off):
                xrh = w_sb[:, _OFF2_XRH + t * _TS:_OFF2_XRH + (t + 1) * _TS]
                xrl = w_sb[:, _OFF2_XRL + t * _TS:_OFF2_XRL + (t + 1) * _TS]
                xch = w_sb[:, _OFF2_XCH + t * _TS:_OFF2_XCH + (t + 1) * _TS]
                xcl = w_sb[:, _OFF2_XCL + t * _TS:_OFF2_XCL + (t + 1) * _TS]
                # bt = G @ xrows[t] (f32 in PSUM), 3-pass hi/lo
                ps_s = pss.tile([16, _TS], f32)
                nc.tensor.matmul(ps_s[:], gh, xrh, start=True, stop=False)
                nc.tensor.matmul(ps_s[:], gh, xrl, start=False, stop=False)
                nc.tensor.matmul(ps_s[:], gl, xrh, start=False, stop=True)
                # split bt into bf16 hi/lo on DVE
                bt_h = btpool.tile([16, _TS], bf16)
                bt_l = btpool.tile([16, _TS], bf16)
                nc.vector.tensor_copy(bt_h[:], ps_s[:])
                nc.vector.tensor_tensor(out=bt_l[:], in0=ps_s[:], in1=bt_h[:],
                                        op=sub)
                # tile(r,c) = bt.T @ xcols[t] in 4 psum chunks, 3-pass each
                for m in range(4):
                    ps = psb.tile([128, _TS], f32)
                    bh_m = bt_h[:, m * 128:(m + 1) * 128]
                    bl_m = bt_l[:, m * 128:(m + 1) * 128]
                    nc.tensor.matmul(ps[:], bh_m, xch, start=True, stop=False)
                    nc.tensor.matmul(ps[:], bh_m, xcl, start=False, stop=False)
                    nc.tensor.matmul(ps[:], bl_m, xch, start=False, stop=True)
                    dst = o_sb[:, off + m * _TS:off + (m + 1) * _TS]
                    if (t * 4 + m) % 2 == 0:
                        nc.vector.tensor_copy(dst, ps[:])
                    else:
                        nc.scalar.copy(dst, ps[:])

            for tp in range(_NT // 2):
                o_sb = opool.tile([128, 2 * 4 * _TS], f32)
                do_tile(2 * tp, o_sb, 0)
                do_tile(2 * tp + 1, o_sb, 4 * _TS)
                nc.sync.dma_start(out=out_d[:, 2 * tp:2 * tp + 2, :], in_=o_sb[:])
            o_last = opool1.tile([128, 4 * _TS], f32)
            do_tile(_NT - 1, o_last, 0)
            nc.sync.dma_start(out=out_d[:, _NT - 1, :], in_=o_last[:])

    _legalize_waits(nc)
    _CACHE["nc2"] = nc
    return nc


def _run2(inputs, trace=False, **kw):
    import ml_dtypes
    from concourse.bass_utils import run_bass_kernel_spmd

    bf = ml_dtypes.bfloat16
    x = np.ascontiguousarray(np.asarray(inputs["input_state"], dtype=np.float32))
    G = _build_G(**{k: v for k, v in inputs.items() if k != "input_state"})
    gh = G.astype(bf)
    gl = (G - gh.astype(np.float32)).astype(bf)
    xh = x.astype(bf)
    xl = (x - xh.astype(np.float32)).astype(bf)

    nc = _get_nc2()
    in_maps = []
    for i in range(_NCORES):
        w = np.empty((16, _W2), dtype=bf)
        w[:, _OFF2_GH:_OFF2_GH + 16] = gh
        w[:, _OFF2_GL:_OFF2_GL + 16] = gl
        for t, (r, c) in enumerate(_tiles_for_core(i)):
            rs, cs = slice(r * _TS, (r + 1) * _TS), slice(c * _TS, (c + 1) * _TS)
            w[:, _OFF2_XRH + t * _TS:_OFF2_XRH + (t + 1) * _TS] = xh[:, rs]
            w[:, _OFF2_XRL + t * _TS:_OFF2_XRL + (t + 1) * _TS] = xl[:, rs]
            w[:, _OFF2_XCH + t * _TS:_OFF2_XCH + (t + 1) * _TS] = xh[:, cs]
            w[:, _OFF2_XCL + t * _TS:_OFF2_XCL + (t + 1) * _TS] = xl[:, cs]
        in_maps.append({"w": w})
    res = run_bass_kernel_spmd(nc, in_maps, core_ids=list(range(_NCORES)),
                               trace=trace, **kw)
    out = np.empty((_B, _B), dtype=np.float32)
    for i in range(_NCORES):
        arr = res.results[i]["out"]  # (128, 17, 2048)
        tiles = np.ascontiguousarray(
            arr.reshape(128, _NT, 4, _TS).transpose(1, 2, 0, 3)
        ).reshape(_NT, _TS, _TS)
        for t, (r, c) in enumerate(_tiles_for_core(i)):
            out[r * _TS:(r + 1) * _TS, c * _TS:(c + 1) * _TS] = tiles[t]
            if c > r:
                out[c * _TS:(c + 1) * _TS, r * _TS:(r + 1) * _TS] = tiles[t].T
    return out, res


_VARIANTS = {"v1": _run, "v2": _run2}


def kernel(**inputs):
    import os
    run = _VARIANTS[os.environ.get("BWN_VARIANT", "v2")]
    out, _ = run(inputs)
    return out


# revision 11
# speedup vs baseline: 2.4723x; 1.3588x over previous
"""Trainium2 Bass kernel for nn_Brick_Wall_Network.

Math: the reference builds a 16x16 complex unitary U from 12 scalars,
computes a = Re(U @ x) (x real => a = Re(U) @ x), and returns
a.T @ (M @ a) with M = I8 (x) Z = diag(+1,-1,...).  Folding:
    out = x.T @ G @ x,   G = Ur.T @ M @ Ur   (16x16 symmetric, Ur = Re(U))
The G build is O(16x16) scalar work replicated on host; all O(B^2)
work runs on the 8 NeuronCores.

v1: row-sharded full output, fp32 matmuls (accuracy-gold baseline).
v2 (default): output is symmetric, so only the 136 upper-triangle
    512x512 tiles are computed (17 per core — perfectly balanced),
    host mirrors the rest.  Matmuls run as bf16 hi/lo 3-pass
    (hi*hi + hi*lo + lo*hi), giving ~1e-5 relative accuracy at the
    bf16 streaming rate (fp32 matmul is ~4x slower per column).

This walrus build encodes at most ONE sync-wait per instruction, while
the Tile scheduler attaches several; _legalize_waits() hoists extras
onto same-engine NoOps.
"""

import numpy as np

_NCORES = 8
_B = 8192
_SH = _B // _NCORES  # 1024 output rows per core (v1)
_CACHE = {}

# v1 packed input layout: [ G (16) | xr (_SH) | x (_B) ] along the free dim
_OFF_G = 0
_OFF_XR = 16
_OFF_X = 16 + _SH
_W = 16 + _SH + _B

# v2: 16 row-blocks of 512; core i owns row-blocks {i, 15-i} => 17 tiles/core
_NT = 17
_TS = 512
# v2 packed bf16 layout: [Gh(16) | Gl(16) | xrh | xrl | xch | xcl]
_L = _NT * _TS
_OFF2_GH = 0
_OFF2_GL = 16
_OFF2_XRH = 32
_OFF2_XRL = 32 + _L
_OFF2_XCH = 32 + 2 * _L
_OFF2_XCL = 32 + 3 * _L
_W2 = 32 + 4 * _L


def _build_G(phi_1, theta_1, omega_1, phi_2, theta_2, omega_2,
             phi_3, theta_3, omega_3, phi_4, theta_4, omega_4):
    def u(phi, theta, omega):
        phi = float(np.asarray(phi).reshape(-1)[0])
        theta = float(np.asarray(theta).reshape(-1)[0])
        omega = float(np.asarray(omega).reshape(-1)[0])
        half = theta / 2.0
        c, s = np.cos(half), np.sin(half)
        return np.array(
            [[c * np.exp(-1j * (phi + omega) / 2), -s * np.exp(1j * (phi - omega) / 2)],
             [s * np.exp(-1j * (phi - omega) / 2), c * np.exp(1j * (phi + omega) / 2)]],
            dtype=np.complex128)

    CNOT = np.array([[1, 0, 0, 0], [0, 1, 0, 0], [0, 0, 0, 1], [0, 0, 1, 0]],
                    dtype=np.float64)
    I2 = np.eye(2, dtype=np.float64)
    Z = np.array([[1.0, 0.0], [0.0, -1.0]], dtype=np.float64)

    g1 = u(phi_1, theta_1, omega_1)
    # NOTE: gate 2 intentionally uses (phi_2, theta_3, omega_4), as in the source.
    g2 = u(phi_2, theta_3, omega_4)
    g3 = u(phi_3, theta_3, omega_3)
    g4 = u(phi_4, theta_4, omega_4)
    layer_1 = np.kron(np.kron(np.kron(g1, g2), g3), g4)

    layer_2 = np.kron(np.kron(CNOT, I2), I2).astype(np.complex128)
    layer_3 = np.kron(np.kron(I2, CNOT), I2).astype(np.complex128)
    l4_real = np.kron(np.kron(I2, I2), CNOT)
    layer_4 = l4_real.astype(np.complex128)
    layer_5 = (l4_real.reshape((2,) * 8).transpose(0, 2, 1, 3, 4, 6, 5, 7)
               .reshape(16, 16).astype(np.complex128))

    U = layer_5 @ (layer_4 @ (layer_3 @ (layer_2 @ layer_1)))
    M = np.kron(np.kron(np.kron(I2, I2), I2), Z)
    Ur = np.real(U)
    G = Ur.T @ (M @ Ur)
    return np.ascontiguousarray(G, dtype=np.float32)


def _legalize_waits(nc):
    """walrus here encodes at most ONE sync-wait per instruction; hoist the
    extras onto same-engine NoOps placed just before the instruction."""
    import concourse.mybir as mybir

    n = 0
    for fn in nc.m.functions:
        for b in fn.blocks:
            new = []
            for inst in b.instructions:
                si = getattr(inst, "sync_info", None)
                ow = list(si.on_wait) if (si is not None and si.on_wait) else []
                if len(ow) > 1:
                    for w in ow[:-1]:
                        nop = mybir.InstNoOp()
                        nop.engine = inst.engine
                        nop.name = f"legal-nop-{n}"
                        nop.sync_info = mybir.SyncInfo(on_wait=[w], on_update=[])
                        new.append(nop)
                        n += 1
                    si.on_wait = [ow[-1]]
                new.append(inst)
            b.instructions[:] = new
    return nc


# ---------------------------------------------------------------------------
# v1: full output, fp32 matmuls
# ---------------------------------------------------------------------------

def _get_nc():
    if "nc" in _CACHE:
        return _CACHE["nc"]
    import concourse.bass as bass
    import concourse.tile as tile
    import concourse.mybir as mybir

    f32 = mybir.dt.float32
    nc = bass.Bass()
    w_d = nc.dram_tensor("w", [16, _W], f32, kind="ExternalInput")
    out_d = nc.dram_tensor("out", [_SH, _B], f32, kind="ExternalOutput")

    with tile.TileContext(nc) as tc:
        with tc.tile_pool(name="const", bufs=1) as cpool, \
             tc.tile_pool(name="outp", bufs=2) as opool, \
             tc.tile_pool(name="ps_small", bufs=2, space="PSUM") as pss, \
             tc.tile_pool(name="ps_big", bufs=6, space="PSUM") as psb:
            w_sb = cpool.tile([16, _W], f32)
            nc.sync.dma_start(out=w_sb[:], in_=w_d[:])
            g_sb = w_sb[:, _OFF_G:_OFF_G + 16]
            xr_sb = w_sb[:, _OFF_XR:_OFF_XR + _SH]
            x_sb = w_sb[:, _OFF_X:_OFF_X + _B]

            # br = G @ xr  (16, _SH); stationary for the main matmuls.
            br_sb = cpool.tile([16, _SH], f32)
            for j in range(_SH // 512):
                ps = pss.tile([16, 512], f32)
                nc.tensor.matmul(ps[:], g_sb, xr_sb[:, j * 512:(j + 1) * 512],
                                 start=True, stop=True)
                nc.vector.tensor_copy(br_sb[:, j * 512:(j + 1) * 512], ps[:])

            # out rows block m: (128, B) = br[:, m*128:...].T @ x
            for m in range(_SH // 128):
                o_sb = opool.tile([128, _B], f32)
                for n in range(_B // 512):
                    ps = psb.tile([128, 512], f32)
                    nc.tensor.matmul(ps[:], br_sb[:, m * 128:(m + 1) * 128],
                                     x_sb[:, n * 512:(n + 1) * 512],
                                     start=True, stop=True)
                    if n % 2 == 0:
                        nc.vector.tensor_copy(o_sb[:, n * 512:(n + 1) * 512], ps[:])
                    else:
                        nc.scalar.copy(o_sb[:, n * 512:(n + 1) * 512], ps[:])
                nc.sync.dma_start(out=out_d[m * 128:(m + 1) * 128, :], in_=o_sb[:])

    _legalize_waits(nc)
    _CACHE["nc"] = nc
    return nc


def _run(inputs, trace=False, **kw):
    from concourse.bass_utils import run_bass_kernel_spmd

    x = np.ascontiguousarray(np.asarray(inputs["input_state"], dtype=np.float32))
    G = _build_G(**{k: v for k, v in inputs.items() if k != "input_state"})
    nc = _get_nc()
    in_maps = []
    for i in range(_NCORES):
        w = np.empty((16, _W), dtype=np.float32)
        w[:, _OFF_G:_OFF_G + 16] = G
        w[:, _OFF_XR:_OFF_XR + _SH] = x[:, i * _SH:(i + 1) * _SH]
        w[:, _OFF_X:_OFF_X + _B] = x
        in_maps.append({"w": w})
    res = run_bass_kernel_spmd(nc, in_maps, core_ids=list(range(_NCORES)),
                               trace=trace, **kw)
    out = np.concatenate([res.results[i]["out"] for i in range(_NCORES)], axis=0)
    return np.ascontiguousarray(out, dtype=np.float32), res


# ---------------------------------------------------------------------------
# v2: symmetric upper-triangle tiles + bf16 hi/lo 3-pass matmuls
# ---------------------------------------------------------------------------

def _tiles_for_core(i):
    r1, r2 = i, 15 - i
    return ([(r1, c) for c in range(r1, 16)] +
            [(r2, c) for c in range(r2, 16)])


def _get_nc2():
    if "nc2" in _CACHE:
        return _CACHE["nc2"]
    import concourse.bass as bass
    import concourse.tile as tile
    import concourse.mybir as mybir

    f32 = mybir.dt.float32
    bf16 = mybir.dt.bfloat16
    sub = mybir.AluOpType.subtract
    nc = bass.Bass()
    w_d = nc.dram_tensor("w", [16, _W2], bf16, kind="ExternalInput")
    # p-major tile store: out[p, t, m*512+j] = tile_t[m*128+p, j]
    out_d = nc.dram_tensor("out", [128, _NT, 4 * _TS], f32, kind="ExternalOutput")

    with tile.TileContext(nc) as tc:
        with tc.tile_pool(name="const", bufs=1) as cpool, \
             tc.tile_pool(name="bt", bufs=3) as btpool, \
             tc.tile_pool(name="outp", bufs=3) as opool, \
             tc.tile_pool(name="outp1", bufs=1) as opool1, \
             tc.tile_pool(name="ps_small", bufs=2, space="PSUM") as pss, \
             tc.tile_pool(name="ps_big", bufs=6, space="PSUM") as psb:
            w_sb = cpool.tile([16, _W2], bf16)
            nc.sync.dma_start(out=w_sb[:], in_=w_d[:])
            gh = w_sb[:, _OFF2_GH:_OFF2_GH + 16]
            gl = w_sb[:, _OFF2_GL:_OFF2_GL + 16]

            def do_tile(t, o_sb, off):
                xrh = w_sb[:, _OFF2_XRH + t * _TS:_OFF2_XRH + (t + 1) * _TS]
                xrl = w_sb[:, _OFF2_XRL + t * _TS:_OFF2_XRL + (t + 1) * _TS]
                xch = w_sb[:, _OFF2_XCH + t * _TS:_OFF2_XCH + (t + 1) * _TS]
                xcl = w_sb[:, _OFF2_XCL + t * _TS:_OFF2_XCL + (t + 1) * _TS]
                # bt = G @ xrows[t] (f32 in PSUM), 3-pass hi/lo
                ps_s = pss.tile([16, _TS], f32)
                nc.tensor.matmul(ps_s[:], gh, xrh, start=True, stop=False)
                nc.tensor.matmul(ps_s[:], gh, xrl, start=False, stop=False)
                nc.tensor.matmul(ps_s[:], gl, xrh, start=False, stop=True)
                # split bt into bf16 hi/lo on DVE
                bt_h = btpool.tile([16, _TS], bf16)
                bt_l = btpool.tile([16, _TS], bf16)
                nc.vector.tensor_copy(bt_h[:], ps_s[:])
                nc.vector.tensor_tensor(out=bt_l[:], in0=ps_s[:], in1=bt_h[:],
                                        op=sub)
                # tile(r,c) = bt.T @ xcols[t] in 4 psum chunks, 3-pass each
                for m in range(4):
                    ps = psb.tile([128, _TS], f32)
                    bh_m = bt_h[:, m * 128:(m + 1) * 128]
                    bl_m = bt_l[:, m * 128:(m + 1) * 128]
                    nc.tensor.matmul(ps[:], bh_m, xch, start=True, stop=False)
                    nc.tensor.matmul(ps[:], bh_m, xcl, start=False, stop=False)
                    nc.tensor.matmul(ps[:], bl_m, xch, start=False, stop=True)
                    dst = o_sb[:, off + m * _TS:off + (m + 1) * _TS]
                    if (t * 4 + m) % 2 == 0:
                        nc.vector.tensor_copy(dst, ps[:])
                    else:
                        nc.scalar.copy(dst, ps[:])

            for tp in range(_NT // 2):
                o_sb = opool.tile([128, 2 * 4 * _TS], f32)
                do_tile(2 * tp, o_sb, 0)
                do_tile(2 * tp + 1, o_sb, 4 * _TS)
                nc.sync.dma_start(out=out_d[:, 2 * tp:2 * tp + 2, :], in_=o_sb[:])
            o_last = opool1.tile([128, 4 * _TS], f32)
            do_tile(_NT - 1, o_last, 0)
            nc.sync.dma_start(out=out_d[:, _NT - 1, :], in_=o_last[:])

    _legalize_waits(nc)
    _CACHE["nc2"] = nc
    return nc


def _run2(inputs, trace=False, **kw):
    import ml_dtypes
    from concourse.bass_utils import run_bass_kernel_spmd

    bf = ml_dtypes.bfloat16
    x = np.ascontiguousarray(np.asarray(inputs["input_state"], dtype=np.float32))
    G = _build_G(**{k: v for k, v in inputs.items() if k != "input_state"})
    gh = G.astype(bf)
    gl = (G - gh.astype(np.float32)).astype(bf)
    xh = x.astype(bf)
    xl = (x - xh.astype(np.float32)).astype(bf)

    nc = _get_nc2()
    in_maps = []
    for i in range(_NCORES):
        w = np.empty((16, _W2), dtype=bf)
        w[:, _OFF2_GH:_OFF2_GH + 16] = gh
        w[:, _OFF2_GL:_OFF2_GL + 16] = gl
        for t, (r, c) in enumerate(_tiles_for_core(i)):
            rs, cs = slice(r * _TS, (r + 1) * _TS), slice(c * _TS, (c + 1) * _TS)
            w[:, _OFF2_XRH + t * _TS:_OFF2_XRH + (t + 1) * _TS] = xh[:, rs]
            w[:, _OFF2_XRL + t * _TS:_OFF2_XRL + (t + 1) * _TS] = xl[:, rs]
            w[:, _OFF2_XCH + t * _TS:_OFF2_XCH + (t + 1) * _TS] = xh[:, cs]
            w[:, _OFF2_XCL + t * _TS:_OFF2_XCL + (t + 1) * _TS] = xl[:, cs]
        in_maps.append({"w": w})
    res = run_bass_kernel_spmd(nc, in_maps, core_ids=list(range(_NCORES)),
                               trace=trace, **kw)
    out = np.empty((_B, _B), dtype=np.float32)
    for i in range(_NCORES):
        arr = res.results[i]["out"]  # (128, 17, 2048)
        tiles = np.ascontiguousarray(
            arr.reshape(128, _NT, 4, _TS).transpose(1, 2, 0, 3)
        ).reshape(_NT, _TS, _TS)
        for t, (r, c) in enumerate(_tiles_for_core(i)):
            out[r * _TS:(r + 1) * _TS, c * _TS:(c + 1) * _TS] = tiles[t]
            if c > r:
                out[c * _TS:(c + 1) * _TS, r * _TS:(r + 1) * _TS] = tiles[t].T
    return out, res


# ---------------------------------------------------------------------------
# v3: v2 + 16-way PE array packing (32x32 tile mode).
# K=16 uses 16 of 128 PE rows; with tile_position the array splits into
# 4x4 independent 32x32 tiles.  The small matmul runs col-tiled with
# zero-padded G weights, producing bt replicated into all 4 row-group
# partition ranges of one PSUM bank; the 4x3 big matmuls per pass then
# run 16-way concurrent (row group = output sub-block m, col group = c).
# Odd row groups keep operands at +16 within their 32-partition window so
# the four xcols replica DMAs land on disjoint SDMA engine sets.
# ---------------------------------------------------------------------------

_DELTA = (0, 0, 0, 0)  # weights/ifmap must start 32-aligned
# v3 wa layout (bf16): [gh0|gh1|gl0|gl1 (32 each) | xrh (L) | xrl (L)]
_OFF3_XRH = 128
_OFF3_XRL = 128 + _L
_W3A = 128 + 2 * _L
_W3C = 2 * _L  # wc layout: [xch (L) | xcl (L)]


def _get_nc3():
    if "nc3" in _CACHE:
        return _CACHE["nc3"]
    import concourse.bass as bass
    import concourse.tile as tile
    import concourse.mybir as mybir

    f32 = mybir.dt.float32
    bf16 = mybir.dt.bfloat16
    sub = mybir.AluOpType.subtract
    nc = bass.Bass()
    wa_d = nc.dram_tensor("wa", [16, _W3A], bf16, kind="ExternalInput")
    wc_d = nc.dram_tensor("wc", [16, _W3C], bf16, kind="ExternalInput")
    out_d = nc.dram_tensor("out", [128, _NT, 4 * _TS], f32, kind="ExternalOutput")

    with tile.TileContext(nc) as tc:
        with tc.tile_pool(name="const", bufs=1) as cpool, \
             tc.tile_pool(name="bt", bufs=3) as btpool, \
             tc.tile_pool(name="outp", bufs=3) as opool, \
             tc.tile_pool(name="outp1", bufs=1) as opool1, \
             tc.tile_pool(name="ps_small", bufs=2, space="PSUM") as pss, \
             tc.tile_pool(name="ps_big", bufs=6, space="PSUM") as psb:
            wa_sb = cpool.tile([16, _W3A], bf16)
            xc_sb = cpool.tile([128, _W3C], bf16)
            nc.sync.dma_start(out=wa_sb[:], in_=wa_d[:])
            # xcols replicated into each row group's operand window; the four
            # destination partition sets map to disjoint SDMA engine sets.
            for g in range(4):
                base = 32 * g + _DELTA[g]
                nc.scalar.dma_start(out=xc_sb[base:base + 16, :], in_=wc_d[:])
            gW = [wa_sb[:, 32 * v:32 * (v + 1)] for v in range(4)]  # gh0 gh1 gl0 gl1

            def do_tile(t, o_sb, off):
                xrh = wa_sb[:, _OFF3_XRH + t * _TS:_OFF3_XRH + (t + 1) * _TS]
                xrl = wa_sb[:, _OFF3_XRL + t * _TS:_OFF3_XRL + (t + 1) * _TS]
                # small: bt = G @ xrows[t], col-tiled -> 4 replicas in one bank,
                # row-group m's replica sits at partitions 32m+delta..+16
                psq = pss.tile([128, _TS], f32)
                for p, (wsel, rh) in enumerate([(0, xrh), (0, xrl), (2, xrh)]):
                    for m in range(4):
                        lhsT = gW[wsel]
                        nc.tensor.matmul(psq[32 * m:32 * m + 32, :], lhsT, rh,
                                         start=(p == 0), stop=(p == 2),
                                         tile_position=(0, 32 * m))
                bq_h = btpool.tile([128, _TS], bf16)
                bq_l = btpool.tile([128, _TS], bf16)
                nc.vector.tensor_copy(bq_h[:], psq[:])
                nc.vector.tensor_tensor(out=bq_l[:], in0=psq[:], in1=bq_h[:],
                                        op=sub)
                # big: 3 passes x 16-way (row group m x col group c)
                ps_m = [psb.tile([128, _TS], f32, name="psm", tag="psm")
                        for _ in range(4)]
                for p in range(3):
                    bq = bq_h if p < 2 else bq_l
                    xoff = _L if p == 1 else 0
                    for m in range(4):
                        base = 32 * m + _DELTA[m]
                        rhs = xc_sb[base:base + 16,
                                    xoff + t * _TS:xoff + (t + 1) * _TS]
                        for c in range(4):
                            lhsT = bq[base:base + 16,
                                      m * 128 + 32 * c:m * 128 + 32 * (c + 1)]
                            nc.tensor.matmul(
                                ps_m[m][32 * c:32 * c + 32, :], lhsT, rhs,
                                start=(p == 0), stop=(p == 2),
                                tile_position=(32 * m, 32 * c))
                for m in range(4):
                    dst = o_sb[:, off + m * _TS:off + (m + 1) * _TS]
                    if (t * 4 + m) % 2 == 0:
                        nc.vector.tensor_copy(dst, ps_m[m][:])
                    else:
                        nc.scalar.copy(dst, ps_m[m][:])

            for tp in range(_NT // 2):
                o_sb = opool.tile([128, 2 * 4 * _TS], f32)
                do_tile(2 * tp, o_sb, 0)
                do_tile(2 * tp + 1, o_sb, 4 * _TS)
                nc.sync.dma_start(out=out_d[:, 2 * tp:2 * tp + 2, :], in_=o_sb[:])
            o_last = opool1.tile([128, 4 * _TS], f32)
            do_tile(_NT - 1, o_last, 0)
            nc.sync.dma_start(out=out_d[:, _NT - 1, :], in_=o_last[:])

    _legalize_waits(nc)
    _CACHE["nc3"] = nc
    return nc


def _run3(inputs, trace=False, **kw):
    import ml_dtypes
    from concourse.bass_utils import run_bass_kernel_spmd

    bf = ml_dtypes.bfloat16
    x = np.ascontiguousarray(np.asarray(inputs["input_state"], dtype=np.float32))
    G = _build_G(**{k: v for k, v in inputs.items() if k != "input_state"})
    gh = G.astype(bf)
    gl = (G - gh.astype(np.float32)).astype(bf)
    xh = x.astype(bf)
    xl = (x - xh.astype(np.float32)).astype(bf)

    nc = _get_nc3()
    in_maps = []
    for i in range(_NCORES):
        wa = np.zeros((16, _W3A), dtype=bf)
        wa[:, 0:16] = gh          # gh0: data in cols 0-15
        wa[:, 48:64] = gh         # gh1: data in cols 16-31 of slot 1
        wa[:, 64:80] = gl         # gl0
        wa[:, 112:128] = gl       # gl1
        wc = np.empty((16, _W3C), dtype=bf)
        for t, (r, c) in enumerate(_tiles_for_core(i)):
            rs, cs = slice(r * _TS, (r + 1) * _TS), slice(c * _TS, (c + 1) * _TS)
            wa[:, _OFF3_XRH + t * _TS:_OFF3_XRH + (t + 1) * _TS] = xh[:, rs]
            wa[:, _OFF3_XRL + t * _TS:_OFF3_XRL + (t + 1) * _TS] = xl[:, rs]
            wc[:, t * _TS:(t + 1) * _TS] = xh[:, cs]
            wc[:, _L + t * _TS:_L + (t + 1) * _TS] = xl[:, cs]
        in_maps.append({"wa": wa, "wc": wc})
    res = run_bass_kernel_spmd(nc, in_maps, core_ids=list(range(_NCORES)),
                               trace=trace, **kw)
    out = np.empty((_B, _B), dtype=np.float32)
    for i in range(_NCORES):
        arr = res.results[i]["out"]  # (128, 17, 2048)
        tiles = np.ascontiguousarray(
            arr.reshape(128, _NT, 4, _TS).transpose(1, 2, 0, 3)
        ).reshape(_NT, _TS, _TS)
        for t, (r, c) in enumerate(_tiles_for_core(i)):
            out[r * _TS:(r + 1) * _TS, c * _TS:(c + 1) * _TS] = tiles[t]
            if c > r:
                out[c * _TS:(c + 1) * _TS, r * _TS:(r + 1) * _TS] = tiles[t].T
    return out, res


_VARIANTS = {"v1": _run, "v2": _run2, "v3": _run3}


def kernel(**inputs):
    import os
    run = _VARIANTS[os.environ.get("BWN_VARIANT", "v2")]
    out, _ = run(inputs)
    return out


# revision 13
# speedup vs baseline: 2.5176x; 1.0183x over previous
"""Trainium2 Bass kernel for nn_Brick_Wall_Network.

Math: the reference builds a 16x16 complex unitary U from 12 scalars,
computes a = Re(U @ x) (x real => a = Re(U) @ x), and returns
a.T @ (M @ a) with M = I8 (x) Z = diag(+1,-1,...).  Folding:
    out = x.T @ G @ x,   G = Ur.T @ M @ Ur   (16x16 symmetric, Ur = Re(U))
The G build is O(16x16) scalar work replicated on host; all O(B^2)
work runs on the 8 NeuronCores.

v1: row-sharded full output, fp32 matmuls (accuracy-gold baseline).
v2 (default): output is symmetric, so only the 136 upper-triangle
    512x512 tiles are computed (17 per core — perfectly balanced),
    host mirrors the rest.  Matmuls run as bf16 hi/lo 3-pass
    (hi*hi + hi*lo + lo*hi), giving ~1e-5 relative accuracy at the
    bf16 streaming rate (fp32 matmul is ~4x slower per column).

This walrus build encodes at most ONE sync-wait per instruction, while
the Tile scheduler attaches several; _legalize_waits() hoists extras
onto same-engine NoOps.
"""

import numpy as np

_NCORES = 8
_B = 8192
_SH = _B // _NCORES  # 1024 output rows per core (v1)
_CACHE = {}

# v1 packed input layout: [ G (16) | xr (_SH) | x (_B) ] along the free dim
_OFF_G = 0
_OFF_XR = 16
_OFF_X = 16 + _SH
_W = 16 + _SH + _B

# v2: 16 row-blocks of 512; core i owns row-blocks {i, 15-i} => 17 tiles/core
_NT = 17
_TS = 512
# v2 packed bf16 layout: [Gh(16) | Gl(16) | xrh | xrl | xch | xcl]
_L = _NT * _TS
_OFF2_GH = 0
_OFF2_GL = 16
_OFF2_XRH = 32
_OFF2_XRL = 32 + _L
_OFF2_XCH = 32 + 2 * _L
_OFF2_XCL = 32 + 3 * _L
_W2 = 32 + 4 * _L


def _build_G(phi_1, theta_1, omega_1, phi_2, theta_2, omega_2,
             phi_3, theta_3, omega_3, phi_4, theta_4, omega_4):
    def u(phi, theta, omega):
        phi = float(np.asarray(phi).reshape(-1)[0])
        theta = float(np.asarray(theta).reshape(-1)[0])
        omega = float(np.asarray(omega).reshape(-1)[0])
        half = theta / 2.0
        c, s = np.cos(half), np.sin(half)
        return np.array(
            [[c * np.exp(-1j * (phi + omega) / 2), -s * np.exp(1j * (phi - omega) / 2)],
             [s * np.exp(-1j * (phi - omega) / 2), c * np.exp(1j * (phi + omega) / 2)]],
            dtype=np.complex128)

    CNOT = np.array([[1, 0, 0, 0], [0, 1, 0, 0], [0, 0, 0, 1], [0, 0, 1, 0]],
                    dtype=np.float64)
    I2 = np.eye(2, dtype=np.float64)
    Z = np.array([[1.0, 0.0], [0.0, -1.0]], dtype=np.float64)

    g1 = u(phi_1, theta_1, omega_1)
    # NOTE: gate 2 intentionally uses (phi_2, theta_3, omega_4), as in the source.
    g2 = u(phi_2, theta_3, omega_4)
    g3 = u(phi_3, theta_3, omega_3)
    g4 = u(phi_4, theta_4, omega_4)
    layer_1 = np.kron(np.kron(np.kron(g1, g2), g3), g4)

    layer_2 = np.kron(np.kron(CNOT, I2), I2).astype(np.complex128)
    layer_3 = np.kron(np.kron(I2, CNOT), I2).astype(np.complex128)
    l4_real = np.kron(np.kron(I2, I2), CNOT)
    layer_4 = l4_real.astype(np.complex128)
    layer_5 = (l4_real.reshape((2,) * 8).transpose(0, 2, 1, 3, 4, 6, 5, 7)
               .reshape(16, 16).astype(np.complex128))

    U = layer_5 @ (layer_4 @ (layer_3 @ (layer_2 @ layer_1)))
    M = np.kron(np.kron(np.kron(I2, I2), I2), Z)
    Ur = np.real(U)
    G = Ur.T @ (M @ Ur)
    return np.ascontiguousarray(G, dtype=np.float32)


def _legalize_waits(nc):
    """walrus here encodes at most ONE sync-wait per instruction; hoist the
    extras onto same-engine NoOps placed just before the instruction."""
    import concourse.mybir as mybir

    n = 0
    for fn in nc.m.functions:
        for b in fn.blocks:
            new = []
            for inst in b.instructions:
                si = getattr(inst, "sync_info", None)
                ow = list(si.on_wait) if (si is not None and si.on_wait) else []
                if len(ow) > 1:
                    for w in ow[:-1]:
                        nop = mybir.InstNoOp()
                        nop.engine = inst.engine
                        nop.name = f"legal-nop-{n}"
                        nop.sync_info = mybir.SyncInfo(on_wait=[w], on_update=[])
                        new.append(nop)
                        n += 1
                    si.on_wait = [ow[-1]]
                new.append(inst)
            b.instructions[:] = new
    return nc


# ---------------------------------------------------------------------------
# v1: full output, fp32 matmuls
# ---------------------------------------------------------------------------

def _get_nc():
    if "nc" in _CACHE:
        return _CACHE["nc"]
    import concourse.bass as bass
    import concourse.tile as tile
    import concourse.mybir as mybir

    f32 = mybir.dt.float32
    nc = bass.Bass()
    w_d = nc.dram_tensor("w", [16, _W], f32, kind="ExternalInput")
    out_d = nc.dram_tensor("out", [_SH, _B], f32, kind="ExternalOutput")

    with tile.TileContext(nc) as tc:
        with tc.tile_pool(name="const", bufs=1) as cpool, \
             tc.tile_pool(name="outp", bufs=2) as opool, \
             tc.tile_pool(name="ps_small", bufs=2, space="PSUM") as pss, \
             tc.tile_pool(name="ps_big", bufs=6, space="PSUM") as psb:
            w_sb = cpool.tile([16, _W], f32)
            nc.sync.dma_start(out=w_sb[:], in_=w_d[:])
            g_sb = w_sb[:, _OFF_G:_OFF_G + 16]
            xr_sb = w_sb[:, _OFF_XR:_OFF_XR + _SH]
            x_sb = w_sb[:, _OFF_X:_OFF_X + _B]

            # br = G @ xr  (16, _SH); stationary for the main matmuls.
            br_sb = cpool.tile([16, _SH], f32)
            for j in range(_SH // 512):
                ps = pss.tile([16, 512], f32)
                nc.tensor.matmul(ps[:], g_sb, xr_sb[:, j * 512:(j + 1) * 512],
                                 start=True, stop=True)
                nc.vector.tensor_copy(br_sb[:, j * 512:(j + 1) * 512], ps[:])

            # out rows block m: (128, B) = br[:, m*128:...].T @ x
            for m in range(_SH // 128):
                o_sb = opool.tile([128, _B], f32)
                for n in range(_B // 512):
                    ps = psb.tile([128, 512], f32)
                    nc.tensor.matmul(ps[:], br_sb[:, m * 128:(m + 1) * 128],
                                     x_sb[:, n * 512:(n + 1) * 512],
                                     start=True, stop=True)
                    if n % 2 == 0:
                        nc.vector.tensor_copy(o_sb[:, n * 512:(n + 1) * 512], ps[:])
                    else:
                        nc.scalar.copy(o_sb[:, n * 512:(n + 1) * 512], ps[:])
                nc.sync.dma_start(out=out_d[m * 128:(m + 1) * 128, :], in_=o_sb[:])

    _legalize_waits(nc)
    _CACHE["nc"] = nc
    return nc


def _run(inputs, trace=False, **kw):
    from concourse.bass_utils import run_bass_kernel_spmd

    x = np.ascontiguousarray(np.asarray(inputs["input_state"], dtype=np.float32))
    G = _build_G(**{k: v for k, v in inputs.items() if k != "input_state"})
    nc = _get_nc()
    in_maps = []
    for i in range(_NCORES):
        w = np.empty((16, _W), dtype=np.float32)
        w[:, _OFF_G:_OFF_G + 16] = G
        w[:, _OFF_XR:_OFF_XR + _SH] = x[:, i * _SH:(i + 1) * _SH]
        w[:, _OFF_X:_OFF_X + _B] = x
        in_maps.append({"w": w})
    res = run_bass_kernel_spmd(nc, in_maps, core_ids=list(range(_NCORES)),
                               trace=trace, **kw)
    out = np.concatenate([res.results[i]["out"] for i in range(_NCORES)], axis=0)
    return np.ascontiguousarray(out, dtype=np.float32), res


# ---------------------------------------------------------------------------
# v2: symmetric upper-triangle tiles + bf16 hi/lo 3-pass matmuls
# ---------------------------------------------------------------------------

def _tiles_for_core(i):
    r1, r2 = i, 15 - i
    return ([(r1, c) for c in range(r1, 16)] +
            [(r2, c) for c in range(r2, 16)])


def _get_nc2():
    if "nc2" in _CACHE:
        return _CACHE["nc2"]
    import concourse.bass as bass
    import concourse.tile as tile
    import concourse.mybir as mybir

    f32 = mybir.dt.float32
    bf16 = mybir.dt.bfloat16
    sub = mybir.AluOpType.subtract
    nc = bass.Bass()
    w_d = nc.dram_tensor("w", [16, _W2], bf16, kind="ExternalInput")
    # p-major tile store: out[p, t, m*512+j] = tile_t[m*128+p, j]
    out_d = nc.dram_tensor("out", [128, _NT, 4 * _TS], f32, kind="ExternalOutput")

    with tile.TileContext(nc) as tc:
        with tc.tile_pool(name="const", bufs=1) as cpool, \
             tc.tile_pool(name="bt", bufs=3) as btpool, \
             tc.tile_pool(name="outp", bufs=3) as opool, \
             tc.tile_pool(name="outp1", bufs=1) as opool1, \
             tc.tile_pool(name="ps_small", bufs=2, space="PSUM") as pss, \
             tc.tile_pool(name="ps_big", bufs=6, space="PSUM") as psb:
            w_sb = cpool.tile([16, _W2], bf16)
            nc.sync.dma_start(out=w_sb[:], in_=w_d[:])
            gh = w_sb[:, _OFF2_GH:_OFF2_GH + 16]
            gl = w_sb[:, _OFF2_GL:_OFF2_GL + 16]

            def do_tile(t, o_sb, off):
                xrh = w_sb[:, _OFF2_XRH + t * _TS:_OFF2_XRH + (t + 1) * _TS]
                xrl = w_sb[:, _OFF2_XRL + t * _TS:_OFF2_XRL + (t + 1) * _TS]
                xch = w_sb[:, _OFF2_XCH + t * _TS:_OFF2_XCH + (t + 1) * _TS]
                xcl = w_sb[:, _OFF2_XCL + t * _TS:_OFF2_XCL + (t + 1) * _TS]
                # bt = G @ xrows[t] (f32 in PSUM), 3-pass hi/lo
                ps_s = pss.tile([16, _TS], f32)
                nc.tensor.matmul(ps_s[:], gh, xrh, start=True, stop=False)
                nc.tensor.matmul(ps_s[:], gh, xrl, start=False, stop=False)
                nc.tensor.matmul(ps_s[:], gl, xrh, start=False, stop=True)
                # split bt into bf16 hi/lo on DVE
                bt_h = btpool.tile([16, _TS], bf16)
                bt_l = btpool.tile([16, _TS], bf16)
                nc.vector.tensor_copy(bt_h[:], ps_s[:])
                nc.vector.tensor_tensor(out=bt_l[:], in0=ps_s[:], in1=bt_h[:],
                                        op=sub)
                # tile(r,c) = bt.T @ xcols[t] in 4 psum chunks, 3-pass each
                for m in range(4):
                    ps = psb.tile([128, _TS], f32)
                    bh_m = bt_h[:, m * 128:(m + 1) * 128]
                    bl_m = bt_l[:, m * 128:(m + 1) * 128]
                    nc.tensor.matmul(ps[:], bh_m, xch, start=True, stop=False)
                    nc.tensor.matmul(ps[:], bh_m, xcl, start=False, stop=False)
                    nc.tensor.matmul(ps[:], bl_m, xch, start=False, stop=True)
                    dst = o_sb[:, off + m * _TS:off + (m + 1) * _TS]
                    if (t * 4 + m) % 2 == 0:
                        nc.vector.tensor_copy(dst, ps[:])
                    else:
                        nc.scalar.copy(dst, ps[:])

            for tp in range(_NT // 2):
                o_sb = opool.tile([128, 2 * 4 * _TS], f32)
                do_tile(2 * tp, o_sb, 0)
                do_tile(2 * tp + 1, o_sb, 4 * _TS)
                nc.sync.dma_start(out=out_d[:, 2 * tp:2 * tp + 2, :], in_=o_sb[:])
            o_last = opool1.tile([128, 4 * _TS], f32)
            do_tile(_NT - 1, o_last, 0)
            nc.sync.dma_start(out=out_d[:, _NT - 1, :], in_=o_last[:])

    _legalize_waits(nc)
    _CACHE["nc2"] = nc
    return nc


def _run2(inputs, trace=False, **kw):
    import ml_dtypes
    from concourse.bass_utils import run_bass_kernel_spmd

    bf = ml_dtypes.bfloat16
    x = np.ascontiguousarray(np.asarray(inputs["input_state"], dtype=np.float32))
    G = _build_G(**{k: v for k, v in inputs.items() if k != "input_state"})
    gh = G.astype(bf)
    gl = (G - gh.astype(np.float32)).astype(bf)
    xh = x.astype(bf)
    xl = (x - xh.astype(np.float32)).astype(bf)

    nc = _get_nc2()
    in_maps = []
    for i in range(_NCORES):
        w = np.empty((16, _W2), dtype=bf)
        w[:, _OFF2_GH:_OFF2_GH + 16] = gh
        w[:, _OFF2_GL:_OFF2_GL + 16] = gl
        for t, (r, c) in enumerate(_tiles_for_core(i)):
            rs, cs = slice(r * _TS, (r + 1) * _TS), slice(c * _TS, (c + 1) * _TS)
            w[:, _OFF2_XRH + t * _TS:_OFF2_XRH + (t + 1) * _TS] = xh[:, rs]
            w[:, _OFF2_XRL + t * _TS:_OFF2_XRL + (t + 1) * _TS] = xl[:, rs]
            w[:, _OFF2_XCH + t * _TS:_OFF2_XCH + (t + 1) * _TS] = xh[:, cs]
            w[:, _OFF2_XCL + t * _TS:_OFF2_XCL + (t + 1) * _TS] = xl[:, cs]
        in_maps.append({"w": w})
    res = run_bass_kernel_spmd(nc, in_maps, core_ids=list(range(_NCORES)),
                               trace=trace, **kw)
    out = np.empty((_B, _B), dtype=np.float32)
    for i in range(_NCORES):
        arr = res.results[i]["out"]  # (128, 17, 2048)
        tiles = np.ascontiguousarray(
            arr.reshape(128, _NT, 4, _TS).transpose(1, 2, 0, 3)
        ).reshape(_NT, _TS, _TS)
        for t, (r, c) in enumerate(_tiles_for_core(i)):
            out[r * _TS:(r + 1) * _TS, c * _TS:(c + 1) * _TS] = tiles[t]
            if c > r:
                out[c * _TS:(c + 1) * _TS, r * _TS:(r + 1) * _TS] = tiles[t].T
    return out, res


# ---------------------------------------------------------------------------
# v3: v2 + 16-way PE array packing (32x32 tile mode).
# K=16 uses 16 of 128 PE rows; with tile_position the array splits into
# 4x4 independent 32x32 tiles.  The small matmul runs col-tiled with
# zero-padded G weights, producing bt replicated into all 4 row-group
# partition ranges of one PSUM bank; the 4x3 big matmuls per pass then
# run 16-way concurrent (row group = output sub-block m, col group = c).
# Odd row groups keep operands at +16 within their 32-partition window so
# the four xcols replica DMAs land on disjoint SDMA engine sets.
# ---------------------------------------------------------------------------

_DELTA = (0, 0, 0, 0)  # weights/ifmap must start 32-aligned
# v3 wa layout (bf16): [gh0|gh1|gl0|gl1 (32 each) | xrh (L) | xrl (L)]
_OFF3_XRH = 128
_OFF3_XRL = 128 + _L
_W3A = 128 + 2 * _L
_W3C = 2 * _L  # wc layout: [xch (L) | xcl (L)]


def _get_nc3():
    if "nc3" in _CACHE:
        return _CACHE["nc3"]
    import concourse.bass as bass
    import concourse.tile as tile
    import concourse.mybir as mybir

    f32 = mybir.dt.float32
    bf16 = mybir.dt.bfloat16
    sub = mybir.AluOpType.subtract
    nc = bass.Bass()
    wa_d = nc.dram_tensor("wa", [16, _W3A], bf16, kind="ExternalInput")
    wc_d = nc.dram_tensor("wc", [16, _W3C], bf16, kind="ExternalInput")
    out_d = nc.dram_tensor("out", [128, _NT, 4 * _TS], f32, kind="ExternalOutput")

    with tile.TileContext(nc) as tc:
        with tc.tile_pool(name="const", bufs=1) as cpool, \
             tc.tile_pool(name="bt", bufs=3) as btpool, \
             tc.tile_pool(name="outp", bufs=3) as opool, \
             tc.tile_pool(name="outp1", bufs=1) as opool1, \
             tc.tile_pool(name="ps_small", bufs=2, space="PSUM") as pss, \
             tc.tile_pool(name="ps_big", bufs=6, space="PSUM") as psb:
            wa_sb = cpool.tile([16, _W3A], bf16)
            xc_sb = cpool.tile([128, _W3C], bf16)
            nc.sync.dma_start(out=wa_sb[:], in_=wa_d[:])
            # xcols replicated into each row group's operand window; the four
            # destination partition sets map to disjoint SDMA engine sets.
            for g in range(4):
                base = 32 * g + _DELTA[g]
                nc.scalar.dma_start(out=xc_sb[base:base + 16, :], in_=wc_d[:])
            gW = [wa_sb[:, 32 * v:32 * (v + 1)] for v in range(4)]  # gh0 gh1 gl0 gl1

            def do_tile(t, o_sb, off):
                xrh = wa_sb[:, _OFF3_XRH + t * _TS:_OFF3_XRH + (t + 1) * _TS]
                xrl = wa_sb[:, _OFF3_XRL + t * _TS:_OFF3_XRL + (t + 1) * _TS]
                # small: bt = G @ xrows[t], col-tiled -> 4 replicas in one bank,
                # row-group m's replica sits at partitions 32m+delta..+16
                psq = pss.tile([128, _TS], f32)
                for p, (wsel, rh) in enumerate([(0, xrh), (0, xrl), (2, xrh)]):
                    for m in range(4):
                        lhsT = gW[wsel]
                        nc.tensor.matmul(psq[32 * m:32 * m + 32, :], lhsT, rh,
                                         start=(p == 0), stop=(p == 2),
                                         tile_position=(0, 32 * m))
                bq_h = btpool.tile([128, _TS], bf16)
                bq_l = btpool.tile([128, _TS], bf16)
                nc.vector.tensor_copy(bq_h[:], psq[:])
                nc.vector.tensor_tensor(out=bq_l[:], in0=psq[:], in1=bq_h[:],
                                        op=sub)
                # big: 3 passes x 16-way (row group m x col group c)
                ps_m = [psb.tile([128, _TS], f32, name="psm", tag="psm")
                        for _ in range(4)]
                for p in range(3):
                    bq = bq_h if p < 2 else bq_l
                    xoff = _L if p == 1 else 0
                    for m in range(4):
                        base = 32 * m + _DELTA[m]
                        rhs = xc_sb[base:base + 16,
                                    xoff + t * _TS:xoff + (t + 1) * _TS]
                        for c in range(4):
                            lhsT = bq[base:base + 16,
                                      m * 128 + 32 * c:m * 128 + 32 * (c + 1)]
                            nc.tensor.matmul(
                                ps_m[m][32 * c:32 * c + 32, :], lhsT, rhs,
                                start=(p == 0), stop=(p == 2),
                                tile_position=(32 * m, 32 * c))
                for m in range(4):
                    dst = o_sb[:, off + m * _TS:off + (m + 1) * _TS]
                    if (t * 4 + m) % 2 == 0:
                        nc.vector.tensor_copy(dst, ps_m[m][:])
                    else:
                        nc.scalar.copy(dst, ps_m[m][:])

            for tp in range(_NT // 2):
                o_sb = opool.tile([128, 2 * 4 * _TS], f32)
                do_tile(2 * tp, o_sb, 0)
                do_tile(2 * tp + 1, o_sb, 4 * _TS)
                nc.sync.dma_start(out=out_d[:, 2 * tp:2 * tp + 2, :], in_=o_sb[:])
            o_last = opool1.tile([128, 4 * _TS], f32)
            do_tile(_NT - 1, o_last, 0)
            nc.sync.dma_start(out=out_d[:, _NT - 1, :], in_=o_last[:])

    _legalize_waits(nc)
    _CACHE["nc3"] = nc
    return nc


def _run3(inputs, trace=False, **kw):
    return _run_packed(_get_nc3, inputs, trace=trace, **kw)


def _run_packed(get_nc, inputs, trace=False, **kw):
    import ml_dtypes
    from concourse.bass_utils import run_bass_kernel_spmd

    bf = ml_dtypes.bfloat16
    x = np.ascontiguousarray(np.asarray(inputs["input_state"], dtype=np.float32))
    G = _build_G(**{k: v for k, v in inputs.items() if k != "input_state"})
    gh = G.astype(bf)
    gl = (G - gh.astype(np.float32)).astype(bf)
    xh = x.astype(bf)
    xl = (x - xh.astype(np.float32)).astype(bf)

    nc = get_nc()
    in_maps = []
    for i in range(_NCORES):
        wa = np.zeros((16, _W3A), dtype=bf)
        wa[:, 0:16] = gh          # gh0: data in cols 0-15
        wa[:, 48:64] = gh         # gh1: data in cols 16-31 of slot 1
        wa[:, 64:80] = gl         # gl0
        wa[:, 112:128] = gl       # gl1
        wc = np.empty((16, _W3C), dtype=bf)
        for t, (r, c) in enumerate(_tiles_for_core(i)):
            rs, cs = slice(r * _TS, (r + 1) * _TS), slice(c * _TS, (c + 1) * _TS)
            wa[:, _OFF3_XRH + t * _TS:_OFF3_XRH + (t + 1) * _TS] = xh[:, rs]
            wa[:, _OFF3_XRL + t * _TS:_OFF3_XRL + (t + 1) * _TS] = xl[:, rs]
            wc[:, t * _TS:(t + 1) * _TS] = xh[:, cs]
            wc[:, _L + t * _TS:_L + (t + 1) * _TS] = xl[:, cs]
        in_maps.append({"wa": wa, "wc": wc})
    res = run_bass_kernel_spmd(nc, in_maps, core_ids=list(range(_NCORES)),
                               trace=trace, **kw)
    out = np.empty((_B, _B), dtype=np.float32)
    for i in range(_NCORES):
        arr = res.results[i]["out"]  # (128, 17, 2048)
        tiles = np.ascontiguousarray(
            arr.reshape(128, _NT, 4, _TS).transpose(1, 2, 0, 3)
        ).reshape(_NT, _TS, _TS)
        for t, (r, c) in enumerate(_tiles_for_core(i)):
            out[r * _TS:(r + 1) * _TS, c * _TS:(c + 1) * _TS] = tiles[t]
            if c > r:
                out[c * _TS:(c + 1) * _TS, r * _TS:(r + 1) * _TS] = tiles[t].T
    return out, res


# ---------------------------------------------------------------------------
# v4: v3 + startup fixes — replica loads issued first in fine interleaved
# chunks (they gate the first big matmuls), wa load split so early tiles'
# xrows arrive before the rest, and the psum->bf16 cast moved to ACT to
# balance DVE/ACT evacuation load.
# ---------------------------------------------------------------------------

def _thin_pe_sem(nc):
    """Tile attaches a PE-sem increment to every PE instruction; each inc
    serializes ~26ns at the end of a concurrent tile_position wave.  Keep
    only increments whose cumulative tick some wait actually references,
    and renumber all PE-sem waits accordingly."""
    blocks = [b for fn in nc.m.functions for b in fn.blocks]
    # the PE engine semaphore: updated by PE-engine instructions
    pe_sems = set()
    for b in blocks:
        for i in b.instructions:
            si = getattr(i, "sync_info", None)
            if si and "PE" in str(i.engine):
                for u in (si.on_update or []):
                    if u.ant_name.startswith("PE"):
                        pe_sems.add(u.ant_name)
    if len(pe_sems) != 1:
        return nc  # unexpected; skip optimization
    sem = next(iter(pe_sems))
    needed = set()
    for b in blocks:
        for i in b.instructions:
            si = getattr(i, "sync_info", None)
            if not si:
                continue
            for w in (si.on_wait or []):
                if w.ant_name == sem:
                    assert w.wait_mode == "sem-ge-imm", w.wait_mode
                    needed.add(w.wait_value)
    remap = {}
    old = new = 0
    for b in blocks:
        for i in b.instructions:
            si = getattr(i, "sync_info", None)
            if not si or not si.on_update:
                continue
            ups = list(si.on_update)
            keep = []
            changed = False
            for u in ups:
                if u.ant_name == sem and u.update_mode == "sem-add-imm":
                    assert u.update_value == 1, u.update_value
                    old += 1
                    if old in needed:
                        new += 1
                        remap[old] = new
                        keep.append(u)
                    else:
                        changed = True
                else:
                    keep.append(u)
            if changed:
                si.on_update = keep
    for b in blocks:
        for i in b.instructions:
            si = getattr(i, "sync_info", None)
            if not si or not si.on_wait:
                continue
            ws = list(si.on_wait)
            changed = False
            for w in ws:
                if w.ant_name == sem:
                    w.wait_value = remap[w.wait_value]
                    changed = True
            if changed:
                si.on_wait = ws
    return nc


def _get_nc4(_thin=False, _key="nc4"):
    if _key in _CACHE:
        return _CACHE[_key]
    import concourse.bass as bass
    import concourse.tile as tile
    import concourse.mybir as mybir

    f32 = mybir.dt.float32
    bf16 = mybir.dt.bfloat16
    sub = mybir.AluOpType.subtract
    nc = bass.Bass()
    wa_d = nc.dram_tensor("wa", [16, _W3A], bf16, kind="ExternalInput")
    wc_d = nc.dram_tensor("wc", [16, _W3C], bf16, kind="ExternalInput")
    out_d = nc.dram_tensor("out", [128, _NT, 4 * _TS], f32, kind="ExternalOutput")

    with tile.TileContext(nc) as tc:
        with tc.tile_pool(name="const", bufs=1) as cpool, \
             tc.tile_pool(name="bt", bufs=3) as btpool, \
             tc.tile_pool(name="outp", bufs=3) as opool, \
             tc.tile_pool(name="outp1", bufs=1) as opool1, \
             tc.tile_pool(name="ps_small", bufs=2, space="PSUM") as pss, \
             tc.tile_pool(name="ps_big", bufs=6, space="PSUM") as psb:
            wa_sb = cpool.tile([16, _W3A], bf16)
            xc_sb = cpool.tile([128, _W3C], bf16)
            # xcols replicas gate the big matmuls: issue them FIRST, in
            # column chunks interleaved across the four row groups so the
            # early tiles' columns land before the rest.
            _NCH = 2
            ch = _L // _NCH
            for k in range(_NCH):
                for g in range(4):
                    base = 32 * g
                    for half in range(2):  # xch, xcl
                        o = half * _L + k * ch
                        nc.scalar.dma_start(out=xc_sb[base:base + 16, o:o + ch],
                                            in_=wc_d[:, o:o + ch])
            # wa: G + early xrows first
            hw = 8 * _TS
            nc.sync.dma_start(out=wa_sb[:, 0:128], in_=wa_d[:, 0:128])
            nc.sync.dma_start(out=wa_sb[:, _OFF3_XRH:_OFF3_XRH + hw],
                              in_=wa_d[:, _OFF3_XRH:_OFF3_XRH + hw])
            nc.sync.dma_start(out=wa_sb[:, _OFF3_XRL:_OFF3_XRL + hw],
                              in_=wa_d[:, _OFF3_XRL:_OFF3_XRL + hw])
            nc.sync.dma_start(out=wa_sb[:, _OFF3_XRH + hw:_OFF3_XRH + _L],
                              in_=wa_d[:, _OFF3_XRH + hw:_OFF3_XRH + _L])
            nc.sync.dma_start(out=wa_sb[:, _OFF3_XRL + hw:_OFF3_XRL + _L],
                              in_=wa_d[:, _OFF3_XRL + hw:_OFF3_XRL + _L])
            gW = [wa_sb[:, 32 * v:32 * (v + 1)] for v in range(4)]

            def do_tile(t, o_sb, off):
                xrh = wa_sb[:, _OFF3_XRH + t * _TS:_OFF3_XRH + (t + 1) * _TS]
                xrl = wa_sb[:, _OFF3_XRL + t * _TS:_OFF3_XRL + (t + 1) * _TS]
                psq = pss.tile([128, _TS], f32)
                for p, (wsel, rh) in enumerate([(0, xrh), (0, xrl), (2, xrh)]):
                    for m in range(4):
                        lhsT = gW[wsel]
                        nc.tensor.matmul(psq[32 * m:32 * m + 32, :], lhsT, rh,
                                         start=(p == 0), stop=(p == 2),
                                         tile_position=(0, 32 * m))
                bq_h = btpool.tile([128, _TS], bf16)
                bq_l = btpool.tile([128, _TS], bf16)
                nc.scalar.copy(bq_h[:], psq[:])
                nc.vector.tensor_tensor(out=bq_l[:], in0=psq[:], in1=bq_h[:],
                                        op=sub)
                ps_m = [psb.tile([128, _TS], f32, name="psm", tag="psm")
                        for _ in range(4)]
                for p in range(3):
                    bq = bq_h if p < 2 else bq_l
                    xoff = _L if p == 1 else 0
                    for m in range(4):
                        base = 32 * m
                        rhs = xc_sb[base:base + 16,
                                    xoff + t * _TS:xoff + (t + 1) * _TS]
                        for c in range(4):
                            lhsT = bq[base:base + 16,
                                      m * 128 + 32 * c:m * 128 + 32 * (c + 1)]
                            nc.tensor.matmul(
                                ps_m[m][32 * c:32 * c + 32, :], lhsT, rhs,
                                start=(p == 0), stop=(p == 2),
                                tile_position=(32 * m, 32 * c))
                for m in range(4):
                    dst = o_sb[:, off + m * _TS:off + (m + 1) * _TS]
                    if (t * 4 + m) % 2 == 0:
                        nc.vector.tensor_copy(dst, ps_m[m][:])
                    else:
                        nc.scalar.copy(dst, ps_m[m][:])

            for tp in range(_NT // 2):
                o_sb = opool.tile([128, 2 * 4 * _TS], f32)
                do_tile(2 * tp, o_sb, 0)
                do_tile(2 * tp + 1, o_sb, 4 * _TS)
                nc.sync.dma_start(out=out_d[:, 2 * tp:2 * tp + 2, :], in_=o_sb[:])
            o_last = opool1.tile([128, 4 * _TS], f32)
            do_tile(_NT - 1, o_last, 0)
            nc.sync.dma_start(out=out_d[:, _NT - 1, :], in_=o_last[:])

    if _thin:
        _thin_pe_sem(nc)
    _legalize_waits(nc)
    _CACHE[_key] = nc
    return nc


def _get_nc5():
    return _get_nc4(_thin=True, _key="nc5")


def _run4(inputs, trace=False, **kw):
    return _run_packed(_get_nc4, inputs, trace=trace, **kw)


def _run5(inputs, trace=False, **kw):
    return _run_packed(_get_nc5, inputs, trace=trace, **kw)


_VARIANTS = {"v1": _run, "v2": _run2, "v3": _run3, "v4": _run4,
             "v5": _run5}


def kernel(**inputs):
    import os
    run = _VARIANTS[os.environ.get("BWN_VARIANT", "v2")]
    out, _ = run(inputs)
    return out


# revision 15
# speedup vs baseline: 2.5580x; 1.0160x over previous
"""Trainium2 Bass kernel for nn_Brick_Wall_Network.

Math: the reference builds a 16x16 complex unitary U from 12 scalars,
computes a = Re(U @ x) (x real => a = Re(U) @ x), and returns
a.T @ (M @ a) with M = I8 (x) Z = diag(+1,-1,...).  Folding:
    out = x.T @ G @ x,   G = Ur.T @ M @ Ur   (16x16 symmetric, Ur = Re(U))
The G build is O(16x16) scalar work replicated on host; all O(B^2)
work runs on the 8 NeuronCores.

v1: row-sharded full output, fp32 matmuls (accuracy-gold baseline).
v2 (default): output is symmetric, so only the 136 upper-triangle
    512x512 tiles are computed (17 per core — perfectly balanced),
    host mirrors the rest.  Matmuls run as bf16 hi/lo 3-pass
    (hi*hi + hi*lo + lo*hi), giving ~1e-5 relative accuracy at the
    bf16 streaming rate (fp32 matmul is ~4x slower per column).

This walrus build encodes at most ONE sync-wait per instruction, while
the Tile scheduler attaches several; _legalize_waits() hoists extras
onto same-engine NoOps.
"""

import numpy as np

_NCORES = 8
_B = 8192
_SH = _B // _NCORES  # 1024 output rows per core (v1)
_CACHE = {}

# v1 packed input layout: [ G (16) | xr (_SH) | x (_B) ] along the free dim
_OFF_G = 0
_OFF_XR = 16
_OFF_X = 16 + _SH
_W = 16 + _SH + _B

# v2: 16 row-blocks of 512; core i owns row-blocks {i, 15-i} => 17 tiles/core
_NT = 17
_TS = 512
# v2 packed bf16 layout: [Gh(16) | Gl(16) | xrh | xrl | xch | xcl]
_L = _NT * _TS
_OFF2_GH = 0
_OFF2_GL = 16
_OFF2_XRH = 32
_OFF2_XRL = 32 + _L
_OFF2_XCH = 32 + 2 * _L
_OFF2_XCL = 32 + 3 * _L
_W2 = 32 + 4 * _L


def _build_G(phi_1, theta_1, omega_1, phi_2, theta_2, omega_2,
             phi_3, theta_3, omega_3, phi_4, theta_4, omega_4):
    def u(phi, theta, omega):
        phi = float(np.asarray(phi).reshape(-1)[0])
        theta = float(np.asarray(theta).reshape(-1)[0])
        omega = float(np.asarray(omega).reshape(-1)[0])
        half = theta / 2.0
        c, s = np.cos(half), np.sin(half)
        return np.array(
            [[c * np.exp(-1j * (phi + omega) / 2), -s * np.exp(1j * (phi - omega) / 2)],
             [s * np.exp(-1j * (phi - omega) / 2), c * np.exp(1j * (phi + omega) / 2)]],
            dtype=np.complex128)

    CNOT = np.array([[1, 0, 0, 0], [0, 1, 0, 0], [0, 0, 0, 1], [0, 0, 1, 0]],
                    dtype=np.float64)
    I2 = np.eye(2, dtype=np.float64)
    Z = np.array([[1.0, 0.0], [0.0, -1.0]], dtype=np.float64)

    g1 = u(phi_1, theta_1, omega_1)
    # NOTE: gate 2 intentionally uses (phi_2, theta_3, omega_4), as in the source.
    g2 = u(phi_2, theta_3, omega_4)
    g3 = u(phi_3, theta_3, omega_3)
    g4 = u(phi_4, theta_4, omega_4)
    layer_1 = np.kron(np.kron(np.kron(g1, g2), g3), g4)

    layer_2 = np.kron(np.kron(CNOT, I2), I2).astype(np.complex128)
    layer_3 = np.kron(np.kron(I2, CNOT), I2).astype(np.complex128)
    l4_real = np.kron(np.kron(I2, I2), CNOT)
    layer_4 = l4_real.astype(np.complex128)
    layer_5 = (l4_real.reshape((2,) * 8).transpose(0, 2, 1, 3, 4, 6, 5, 7)
               .reshape(16, 16).astype(np.complex128))

    U = layer_5 @ (layer_4 @ (layer_3 @ (layer_2 @ layer_1)))
    M = np.kron(np.kron(np.kron(I2, I2), I2), Z)
    Ur = np.real(U)
    G = Ur.T @ (M @ Ur)
    return np.ascontiguousarray(G, dtype=np.float32)


def _legalize_waits(nc):
    """walrus here encodes at most ONE sync-wait per instruction; hoist the
    extras onto same-engine NoOps placed just before the instruction."""
    import concourse.mybir as mybir

    n = 0
    for fn in nc.m.functions:
        for b in fn.blocks:
            new = []
            for inst in b.instructions:
                si = getattr(inst, "sync_info", None)
                ow = list(si.on_wait) if (si is not None and si.on_wait) else []
                if len(ow) > 1:
                    for w in ow[:-1]:
                        nop = mybir.InstNoOp()
                        nop.engine = inst.engine
                        nop.name = f"legal-nop-{n}"
                        nop.sync_info = mybir.SyncInfo(on_wait=[w], on_update=[])
                        new.append(nop)
                        n += 1
                    si.on_wait = [ow[-1]]
                new.append(inst)
            b.instructions[:] = new
    return nc


# ---------------------------------------------------------------------------
# v1: full output, fp32 matmuls
# ---------------------------------------------------------------------------

def _get_nc():
    if "nc" in _CACHE:
        return _CACHE["nc"]
    import concourse.bass as bass
    import concourse.tile as tile
    import concourse.mybir as mybir

    f32 = mybir.dt.float32
    nc = bass.Bass()
    w_d = nc.dram_tensor("w", [16, _W], f32, kind="ExternalInput")
    out_d = nc.dram_tensor("out", [_SH, _B], f32, kind="ExternalOutput")

    with tile.TileContext(nc) as tc:
        with tc.tile_pool(name="const", bufs=1) as cpool, \
             tc.tile_pool(name="outp", bufs=2) as opool, \
             tc.tile_pool(name="ps_small", bufs=2, space="PSUM") as pss, \
             tc.tile_pool(name="ps_big", bufs=6, space="PSUM") as psb:
            w_sb = cpool.tile([16, _W], f32)
            nc.sync.dma_start(out=w_sb[:], in_=w_d[:])
            g_sb = w_sb[:, _OFF_G:_OFF_G + 16]
            xr_sb = w_sb[:, _OFF_XR:_OFF_XR + _SH]
            x_sb = w_sb[:, _OFF_X:_OFF_X + _B]

            # br = G @ xr  (16, _SH); stationary for the main matmuls.
            br_sb = cpool.tile([16, _SH], f32)
            for j in range(_SH // 512):
                ps = pss.tile([16, 512], f32)
                nc.tensor.matmul(ps[:], g_sb, xr_sb[:, j * 512:(j + 1) * 512],
                                 start=True, stop=True)
                nc.vector.tensor_copy(br_sb[:, j * 512:(j + 1) * 512], ps[:])

            # out rows block m: (128, B) = br[:, m*128:...].T @ x
            for m in range(_SH // 128):
                o_sb = opool.tile([128, _B], f32)
                for n in range(_B // 512):
                    ps = psb.tile([128, 512], f32)
                    nc.tensor.matmul(ps[:], br_sb[:, m * 128:(m + 1) * 128],
                                     x_sb[:, n * 512:(n + 1) * 512],
                                     start=True, stop=True)
                    if n % 2 == 0:
                        nc.vector.tensor_copy(o_sb[:, n * 512:(n + 1) * 512], ps[:])
                    else:
                        nc.scalar.copy(o_sb[:, n * 512:(n + 1) * 512], ps[:])
                nc.sync.dma_start(out=out_d[m * 128:(m + 1) * 128, :], in_=o_sb[:])

    _legalize_waits(nc)
    _CACHE["nc"] = nc
    return nc


def _run(inputs, trace=False, **kw):
    from concourse.bass_utils import run_bass_kernel_spmd

    x = np.ascontiguousarray(np.asarray(inputs["input_state"], dtype=np.float32))
    G = _build_G(**{k: v for k, v in inputs.items() if k != "input_state"})
    nc = _get_nc()
    in_maps = []
    for i in range(_NCORES):
        w = np.empty((16, _W), dtype=np.float32)
        w[:, _OFF_G:_OFF_G + 16] = G
        w[:, _OFF_XR:_OFF_XR + _SH] = x[:, i * _SH:(i + 1) * _SH]
        w[:, _OFF_X:_OFF_X + _B] = x
        in_maps.append({"w": w})
    res = run_bass_kernel_spmd(nc, in_maps, core_ids=list(range(_NCORES)),
                               trace=trace, **kw)
    out = np.concatenate([res.results[i]["out"] for i in range(_NCORES)], axis=0)
    return np.ascontiguousarray(out, dtype=np.float32), res


# ---------------------------------------------------------------------------
# v2: symmetric upper-triangle tiles + bf16 hi/lo 3-pass matmuls
# ---------------------------------------------------------------------------

def _tiles_for_core(i):
    r1, r2 = i, 15 - i
    return ([(r1, c) for c in range(r1, 16)] +
            [(r2, c) for c in range(r2, 16)])


def _get_nc2():
    if "nc2" in _CACHE:
        return _CACHE["nc2"]
    import concourse.bass as bass
    import concourse.tile as tile
    import concourse.mybir as mybir

    f32 = mybir.dt.float32
    bf16 = mybir.dt.bfloat16
    sub = mybir.AluOpType.subtract
    nc = bass.Bass()
    w_d = nc.dram_tensor("w", [16, _W2], bf16, kind="ExternalInput")
    # p-major tile store: out[p, t, m*512+j] = tile_t[m*128+p, j]
    out_d = nc.dram_tensor("out", [128, _NT, 4 * _TS], f32, kind="ExternalOutput")

    with tile.TileContext(nc) as tc:
        with tc.tile_pool(name="const", bufs=1) as cpool, \
             tc.tile_pool(name="bt", bufs=3) as btpool, \
             tc.tile_pool(name="outp", bufs=3) as opool, \
             tc.tile_pool(name="outp1", bufs=1) as opool1, \
             tc.tile_pool(name="ps_small", bufs=2, space="PSUM") as pss, \
             tc.tile_pool(name="ps_big", bufs=6, space="PSUM") as psb:
            w_sb = cpool.tile([16, _W2], bf16)
            nc.sync.dma_start(out=w_sb[:], in_=w_d[:])
            gh = w_sb[:, _OFF2_GH:_OFF2_GH + 16]
            gl = w_sb[:, _OFF2_GL:_OFF2_GL + 16]

            def do_tile(t, o_sb, off):
                xrh = w_sb[:, _OFF2_XRH + t * _TS:_OFF2_XRH + (t + 1) * _TS]
                xrl = w_sb[:, _OFF2_XRL + t * _TS:_OFF2_XRL + (t + 1) * _TS]
                xch = w_sb[:, _OFF2_XCH + t * _TS:_OFF2_XCH + (t + 1) * _TS]
                xcl = w_sb[:, _OFF2_XCL + t * _TS:_OFF2_XCL + (t + 1) * _TS]
                # bt = G @ xrows[t] (f32 in PSUM), 3-pass hi/lo
                ps_s = pss.tile([16, _TS], f32)
                nc.tensor.matmul(ps_s[:], gh, xrh, start=True, stop=False)
                nc.tensor.matmul(ps_s[:], gh, xrl, start=False, stop=False)
                nc.tensor.matmul(ps_s[:], gl, xrh, start=False, stop=True)
                # split bt into bf16 hi/lo on DVE
                bt_h = btpool.tile([16, _TS], bf16)
                bt_l = btpool.tile([16, _TS], bf16)
                nc.vector.tensor_copy(bt_h[:], ps_s[:])
                nc.vector.tensor_tensor(out=bt_l[:], in0=ps_s[:], in1=bt_h[:],
                                        op=sub)
                # tile(r,c) = bt.T @ xcols[t] in 4 psum chunks, 3-pass each
                for m in range(4):
                    ps = psb.tile([128, _TS], f32)
                    bh_m = bt_h[:, m * 128:(m + 1) * 128]
                    bl_m = bt_l[:, m * 128:(m + 1) * 128]
                    nc.tensor.matmul(ps[:], bh_m, xch, start=True, stop=False)
                    nc.tensor.matmul(ps[:], bh_m, xcl, start=False, stop=False)
                    nc.tensor.matmul(ps[:], bl_m, xch, start=False, stop=True)
                    dst = o_sb[:, off + m * _TS:off + (m + 1) * _TS]
                    if (t * 4 + m) % 2 == 0:
                        nc.vector.tensor_copy(dst, ps[:])
                    else:
                        nc.scalar.copy(dst, ps[:])

            for tp in range(_NT // 2):
                o_sb = opool.tile([128, 2 * 4 * _TS], f32)
                do_tile(2 * tp, o_sb, 0)
                do_tile(2 * tp + 1, o_sb, 4 * _TS)
                nc.sync.dma_start(out=out_d[:, 2 * tp:2 * tp + 2, :], in_=o_sb[:])
            o_last = opool1.tile([128, 4 * _TS], f32)
            do_tile(_NT - 1, o_last, 0)
            nc.sync.dma_start(out=out_d[:, _NT - 1, :], in_=o_last[:])

    _legalize_waits(nc)
    _CACHE["nc2"] = nc
    return nc


def _run2(inputs, trace=False, **kw):
    import ml_dtypes
    from concourse.bass_utils import run_bass_kernel_spmd

    bf = ml_dtypes.bfloat16
    x = np.ascontiguousarray(np.asarray(inputs["input_state"], dtype=np.float32))
    G = _build_G(**{k: v for k, v in inputs.items() if k != "input_state"})
    gh = G.astype(bf)
    gl = (G - gh.astype(np.float32)).astype(bf)
    xh = x.astype(bf)
    xl = (x - xh.astype(np.float32)).astype(bf)

    nc = _get_nc2()
    in_maps = []
    for i in range(_NCORES):
        w = np.empty((16, _W2), dtype=bf)
        w[:, _OFF2_GH:_OFF2_GH + 16] = gh
        w[:, _OFF2_GL:_OFF2_GL + 16] = gl
        for t, (r, c) in enumerate(_tiles_for_core(i)):
            rs, cs = slice(r * _TS, (r + 1) * _TS), slice(c * _TS, (c + 1) * _TS)
            w[:, _OFF2_XRH + t * _TS:_OFF2_XRH + (t + 1) * _TS] = xh[:, rs]
            w[:, _OFF2_XRL + t * _TS:_OFF2_XRL + (t + 1) * _TS] = xl[:, rs]
            w[:, _OFF2_XCH + t * _TS:_OFF2_XCH + (t + 1) * _TS] = xh[:, cs]
            w[:, _OFF2_XCL + t * _TS:_OFF2_XCL + (t + 1) * _TS] = xl[:, cs]
        in_maps.append({"w": w})
    res = run_bass_kernel_spmd(nc, in_maps, core_ids=list(range(_NCORES)),
                               trace=trace, **kw)
    out = np.empty((_B, _B), dtype=np.float32)
    for i in range(_NCORES):
        arr = res.results[i]["out"]  # (128, 17, 2048)
        tiles = np.ascontiguousarray(
            arr.reshape(128, _NT, 4, _TS).transpose(1, 2, 0, 3)
        ).reshape(_NT, _TS, _TS)
        for t, (r, c) in enumerate(_tiles_for_core(i)):
            out[r * _TS:(r + 1) * _TS, c * _TS:(c + 1) * _TS] = tiles[t]
            if c > r:
                out[c * _TS:(c + 1) * _TS, r * _TS:(r + 1) * _TS] = tiles[t].T
    return out, res


# ---------------------------------------------------------------------------
# v3: v2 + 16-way PE array packing (32x32 tile mode).
# K=16 uses 16 of 128 PE rows; with tile_position the array splits into
# 4x4 independent 32x32 tiles.  The small matmul runs col-tiled with
# zero-padded G weights, producing bt replicated into all 4 row-group
# partition ranges of one PSUM bank; the 4x3 big matmuls per pass then
# run 16-way concurrent (row group = output sub-block m, col group = c).
# Odd row groups keep operands at +16 within their 32-partition window so
# the four xcols replica DMAs land on disjoint SDMA engine sets.
# ---------------------------------------------------------------------------

_DELTA = (0, 0, 0, 0)  # weights/ifmap must start 32-aligned
# v3 wa layout (bf16): [gh0|gh1|gl0|gl1 (32 each) | xrh (L) | xrl (L)]
_OFF3_XRH = 128
_OFF3_XRL = 128 + _L
_W3A = 128 + 2 * _L
_W3C = 2 * _L  # wc layout: [xch (L) | xcl (L)]


def _get_nc3():
    if "nc3" in _CACHE:
        return _CACHE["nc3"]
    import concourse.bass as bass
    import concourse.tile as tile
    import concourse.mybir as mybir

    f32 = mybir.dt.float32
    bf16 = mybir.dt.bfloat16
    sub = mybir.AluOpType.subtract
    nc = bass.Bass()
    wa_d = nc.dram_tensor("wa", [16, _W3A], bf16, kind="ExternalInput")
    wc_d = nc.dram_tensor("wc", [16, _W3C], bf16, kind="ExternalInput")
    out_d = nc.dram_tensor("out", [128, _NT, 4 * _TS], f32, kind="ExternalOutput")

    with tile.TileContext(nc) as tc:
        with tc.tile_pool(name="const", bufs=1) as cpool, \
             tc.tile_pool(name="bt", bufs=3) as btpool, \
             tc.tile_pool(name="outp", bufs=3) as opool, \
             tc.tile_pool(name="outp1", bufs=1) as opool1, \
             tc.tile_pool(name="ps_small", bufs=2, space="PSUM") as pss, \
             tc.tile_pool(name="ps_big", bufs=6, space="PSUM") as psb:
            wa_sb = cpool.tile([16, _W3A], bf16)
            xc_sb = cpool.tile([128, _W3C], bf16)
            nc.sync.dma_start(out=wa_sb[:], in_=wa_d[:])
            # xcols replicated into each row group's operand window; the four
            # destination partition sets map to disjoint SDMA engine sets.
            for g in range(4):
                base = 32 * g + _DELTA[g]
                nc.scalar.dma_start(out=xc_sb[base:base + 16, :], in_=wc_d[:])
            gW = [wa_sb[:, 32 * v:32 * (v + 1)] for v in range(4)]  # gh0 gh1 gl0 gl1

            def do_tile(t, o_sb, off):
                xrh = wa_sb[:, _OFF3_XRH + t * _TS:_OFF3_XRH + (t + 1) * _TS]
                xrl = wa_sb[:, _OFF3_XRL + t * _TS:_OFF3_XRL + (t + 1) * _TS]
                # small: bt = G @ xrows[t], col-tiled -> 4 replicas in one bank,
                # row-group m's replica sits at partitions 32m+delta..+16
                psq = pss.tile([128, _TS], f32)
                for p, (wsel, rh) in enumerate([(0, xrh), (0, xrl), (2, xrh)]):
                    for m in range(4):
                        lhsT = gW[wsel]
                        nc.tensor.matmul(psq[32 * m:32 * m + 32, :], lhsT, rh,
                                         start=(p == 0), stop=(p == 2),
                                         tile_position=(0, 32 * m))
                bq_h = btpool.tile([128, _TS], bf16)
                bq_l = btpool.tile([128, _TS], bf16)
                nc.vector.tensor_copy(bq_h[:], psq[:])
                nc.vector.tensor_tensor(out=bq_l[:], in0=psq[:], in1=bq_h[:],
                                        op=sub)
                # big: 3 passes x 16-way (row group m x col group c)
                ps_m = [psb.tile([128, _TS], f32, name="psm", tag="psm")
                        for _ in range(4)]
                for p in range(3):
                    bq = bq_h if p < 2 else bq_l
                    xoff = _L if p == 1 else 0
                    for m in range(4):
                        base = 32 * m + _DELTA[m]
                        rhs = xc_sb[base:base + 16,
                                    xoff + t * _TS:xoff + (t + 1) * _TS]
                        for c in range(4):
                            lhsT = bq[base:base + 16,
                                      m * 128 + 32 * c:m * 128 + 32 * (c + 1)]
                            nc.tensor.matmul(
                                ps_m[m][32 * c:32 * c + 32, :], lhsT, rhs,
                                start=(p == 0), stop=(p == 2),
                                tile_position=(32 * m, 32 * c))
                for m in range(4):
                    dst = o_sb[:, off + m * _TS:off + (m + 1) * _TS]
                    if (t * 4 + m) % 2 == 0:
                        nc.vector.tensor_copy(dst, ps_m[m][:])
                    else:
                        nc.scalar.copy(dst, ps_m[m][:])

            for tp in range(_NT // 2):
                o_sb = opool.tile([128, 2 * 4 * _TS], f32)
                do_tile(2 * tp, o_sb, 0)
                do_tile(2 * tp + 1, o_sb, 4 * _TS)
                nc.sync.dma_start(out=out_d[:, 2 * tp:2 * tp + 2, :], in_=o_sb[:])
            o_last = opool1.tile([128, 4 * _TS], f32)
            do_tile(_NT - 1, o_last, 0)
            nc.sync.dma_start(out=out_d[:, _NT - 1, :], in_=o_last[:])

    _legalize_waits(nc)
    _CACHE["nc3"] = nc
    return nc


def _run3(inputs, trace=False, **kw):
    return _run_packed(_get_nc3, inputs, trace=trace, **kw)


def _run_packed(get_nc, inputs, trace=False, **kw):
    import ml_dtypes
    from concourse.bass_utils import run_bass_kernel_spmd

    bf = ml_dtypes.bfloat16
    x = np.ascontiguousarray(np.asarray(inputs["input_state"], dtype=np.float32))
    G = _build_G(**{k: v for k, v in inputs.items() if k != "input_state"})
    gh = G.astype(bf)
    gl = (G - gh.astype(np.float32)).astype(bf)
    xh = x.astype(bf)
    xl = (x - xh.astype(np.float32)).astype(bf)

    nc = get_nc()
    in_maps = []
    for i in range(_NCORES):
        wa = np.zeros((16, _W3A), dtype=bf)
        wa[:, 0:16] = gh          # gh0: data in cols 0-15
        wa[:, 48:64] = gh         # gh1: data in cols 16-31 of slot 1
        wa[:, 64:80] = gl         # gl0
        wa[:, 112:128] = gl       # gl1
        wc = np.empty((16, _W3C), dtype=bf)
        for t, (r, c) in enumerate(_tiles_for_core(i)):
            rs, cs = slice(r * _TS, (r + 1) * _TS), slice(c * _TS, (c + 1) * _TS)
            wa[:, _OFF3_XRH + t * _TS:_OFF3_XRH + (t + 1) * _TS] = xh[:, rs]
            wa[:, _OFF3_XRL + t * _TS:_OFF3_XRL + (t + 1) * _TS] = xl[:, rs]
            wc[:, t * _TS:(t + 1) * _TS] = xh[:, cs]
            wc[:, _L + t * _TS:_L + (t + 1) * _TS] = xl[:, cs]
        in_maps.append({"wa": wa, "wc": wc})
    res = run_bass_kernel_spmd(nc, in_maps, core_ids=list(range(_NCORES)),
                               trace=trace, **kw)
    out = np.empty((_B, _B), dtype=np.float32)
    for i in range(_NCORES):
        arr = res.results[i]["out"]  # (128, 17, 2048)
        tiles = np.ascontiguousarray(
            arr.reshape(128, _NT, 4, _TS).transpose(1, 2, 0, 3)
        ).reshape(_NT, _TS, _TS)
        for t, (r, c) in enumerate(_tiles_for_core(i)):
            out[r * _TS:(r + 1) * _TS, c * _TS:(c + 1) * _TS] = tiles[t]
            if c > r:
                out[c * _TS:(c + 1) * _TS, r * _TS:(r + 1) * _TS] = tiles[t].T
    return out, res


# ---------------------------------------------------------------------------
# v4: v3 + startup fixes — replica loads issued first in fine interleaved
# chunks (they gate the first big matmuls), wa load split so early tiles'
# xrows arrive before the rest, and the psum->bf16 cast moved to ACT to
# balance DVE/ACT evacuation load.
# ---------------------------------------------------------------------------

def _thin_pe_sem(nc):
    """Tile attaches a PE-sem increment to every PE instruction; each inc
    serializes ~26ns at the end of a concurrent tile_position wave.  Keep
    only increments whose cumulative tick some wait actually references,
    and renumber all PE-sem waits accordingly."""
    blocks = [b for fn in nc.m.functions for b in fn.blocks]
    # the PE engine semaphore: updated by PE-engine instructions
    pe_sems = set()
    for b in blocks:
        for i in b.instructions:
            si = getattr(i, "sync_info", None)
            if si and "PE" in str(i.engine):
                for u in (si.on_update or []):
                    if u.ant_name.startswith("PE"):
                        pe_sems.add(u.ant_name)
    if len(pe_sems) != 1:
        return nc  # unexpected; skip optimization
    sem = next(iter(pe_sems))
    needed = set()
    for b in blocks:
        for i in b.instructions:
            si = getattr(i, "sync_info", None)
            if not si:
                continue
            for w in (si.on_wait or []):
                if w.ant_name == sem:
                    assert w.wait_mode == "sem-ge-imm", w.wait_mode
                    needed.add(w.wait_value)
    remap = {}
    old = new = 0
    for b in blocks:
        for i in b.instructions:
            si = getattr(i, "sync_info", None)
            if not si or not si.on_update:
                continue
            ups = list(si.on_update)
            keep = []
            changed = False
            for u in ups:
                if u.ant_name == sem and u.update_mode in ("sem-inc", "sem-add-imm"):
                    assert (u.update_value or 1) == 1, u.update_value
                    old += 1
                    if old in needed:
                        new += 1
                        remap[old] = new
                        keep.append(u)
                    else:
                        changed = True
                else:
                    keep.append(u)
            if changed:
                si.on_update = keep
    seen = set()
    for b in blocks:
        for i in b.instructions:
            si = getattr(i, "sync_info", None)
            if not si or not si.on_wait:
                continue
            ws = list(si.on_wait)
            changed = False
            for w in ws:
                if w.ant_name == sem and id(w) not in seen:
                    seen.add(id(w))
                    w.wait_value = remap[w.wait_value]
                    changed = True
            if changed:
                si.on_wait = ws
    return nc


def _get_nc4(_thin=False, _key="nc4"):
    if _key in _CACHE:
        return _CACHE[_key]
    import concourse.bass as bass
    import concourse.tile as tile
    import concourse.mybir as mybir

    f32 = mybir.dt.float32
    bf16 = mybir.dt.bfloat16
    sub = mybir.AluOpType.subtract
    nc = bass.Bass()
    wa_d = nc.dram_tensor("wa", [16, _W3A], bf16, kind="ExternalInput")
    wc_d = nc.dram_tensor("wc", [16, _W3C], bf16, kind="ExternalInput")
    out_d = nc.dram_tensor("out", [128, _NT, 4 * _TS], f32, kind="ExternalOutput")

    with tile.TileContext(nc) as tc:
        with tc.tile_pool(name="const", bufs=1) as cpool, \
             tc.tile_pool(name="bt", bufs=3) as btpool, \
             tc.tile_pool(name="outp", bufs=3) as opool, \
             tc.tile_pool(name="outp1", bufs=1) as opool1, \
             tc.tile_pool(name="ps_small", bufs=2, space="PSUM") as pss, \
             tc.tile_pool(name="ps_big", bufs=6, space="PSUM") as psb:
            wa_sb = cpool.tile([16, _W3A], bf16)
            xc_sb = cpool.tile([128, _W3C], bf16)
            # xcols replicas gate the big matmuls: issue them FIRST, in
            # column chunks interleaved across the four row groups so the
            # early tiles' columns land before the rest.
            _NCH = 2
            ch = _L // _NCH
            for k in range(_NCH):
                for g in range(4):
                    base = 32 * g
                    for half in range(2):  # xch, xcl
                        o = half * _L + k * ch
                        nc.scalar.dma_start(out=xc_sb[base:base + 16, o:o + ch],
                                            in_=wc_d[:, o:o + ch])
            # wa: G + early xrows first
            hw = 8 * _TS
            nc.sync.dma_start(out=wa_sb[:, 0:128], in_=wa_d[:, 0:128])
            nc.sync.dma_start(out=wa_sb[:, _OFF3_XRH:_OFF3_XRH + hw],
                              in_=wa_d[:, _OFF3_XRH:_OFF3_XRH + hw])
            nc.sync.dma_start(out=wa_sb[:, _OFF3_XRL:_OFF3_XRL + hw],
                              in_=wa_d[:, _OFF3_XRL:_OFF3_XRL + hw])
            nc.sync.dma_start(out=wa_sb[:, _OFF3_XRH + hw:_OFF3_XRH + _L],
                              in_=wa_d[:, _OFF3_XRH + hw:_OFF3_XRH + _L])
            nc.sync.dma_start(out=wa_sb[:, _OFF3_XRL + hw:_OFF3_XRL + _L],
                              in_=wa_d[:, _OFF3_XRL + hw:_OFF3_XRL + _L])
            gW = [wa_sb[:, 32 * v:32 * (v + 1)] for v in range(4)]

            def do_tile(t, o_sb, off):
                xrh = wa_sb[:, _OFF3_XRH + t * _TS:_OFF3_XRH + (t + 1) * _TS]
                xrl = wa_sb[:, _OFF3_XRL + t * _TS:_OFF3_XRL + (t + 1) * _TS]
                psq = pss.tile([128, _TS], f32)
                for p, (wsel, rh) in enumerate([(0, xrh), (0, xrl), (2, xrh)]):
                    for m in range(4):
                        lhsT = gW[wsel]
                        nc.tensor.matmul(psq[32 * m:32 * m + 32, :], lhsT, rh,
                                         start=(p == 0), stop=(p == 2),
                                         tile_position=(0, 32 * m))
                bq_h = btpool.tile([128, _TS], bf16)
                bq_l = btpool.tile([128, _TS], bf16)
                nc.scalar.copy(bq_h[:], psq[:])
                nc.vector.tensor_tensor(out=bq_l[:], in0=psq[:], in1=bq_h[:],
                                        op=sub)
                ps_m = [psb.tile([128, _TS], f32, name="psm", tag="psm")
                        for _ in range(4)]
                for p in range(3):
                    bq = bq_h if p < 2 else bq_l
                    xoff = _L if p == 1 else 0
                    for m in range(4):
                        base = 32 * m
                        rhs = xc_sb[base:base + 16,
                                    xoff + t * _TS:xoff + (t + 1) * _TS]
                        for c in range(4):
                            lhsT = bq[base:base + 16,
                                      m * 128 + 32 * c:m * 128 + 32 * (c + 1)]
                            nc.tensor.matmul(
                                ps_m[m][32 * c:32 * c + 32, :], lhsT, rhs,
                                start=(p == 0), stop=(p == 2),
                                tile_position=(32 * m, 32 * c))
                for m in range(4):
                    dst = o_sb[:, off + m * _TS:off + (m + 1) * _TS]
                    if (t * 4 + m) % 2 == 0:
                        nc.vector.tensor_copy(dst, ps_m[m][:])
                    else:
                        nc.scalar.copy(dst, ps_m[m][:])

            for tp in range(_NT // 2):
                o_sb = opool.tile([128, 2 * 4 * _TS], f32)
                do_tile(2 * tp, o_sb, 0)
                do_tile(2 * tp + 1, o_sb, 4 * _TS)
                nc.sync.dma_start(out=out_d[:, 2 * tp:2 * tp + 2, :], in_=o_sb[:])
            o_last = opool1.tile([128, 4 * _TS], f32)
            do_tile(_NT - 1, o_last, 0)
            nc.sync.dma_start(out=out_d[:, _NT - 1, :], in_=o_last[:])

    if _thin:
        _thin_pe_sem(nc)
    _legalize_waits(nc)
    _CACHE[_key] = nc
    return nc


def _get_nc5():
    return _get_nc4(_thin=True, _key="nc5")


def _run4(inputs, trace=False, **kw):
    return _run_packed(_get_nc4, inputs, trace=trace, **kw)


def _run5(inputs, trace=False, **kw):
    return _run_packed(_get_nc5, inputs, trace=trace, **kw)


_VARIANTS = {"v1": _run, "v2": _run2, "v3": _run3, "v4": _run4,
             "v5": _run5}


def kernel(**inputs):
    import os
    run = _VARIANTS[os.environ.get("BWN_VARIANT", "v2")]
    out, _ = run(inputs)
    return out


# revision 17
# speedup vs baseline: 2.8844x; 1.1276x over previous
"""Trainium2 Bass kernel for nn_Brick_Wall_Network.

Math: the reference builds a 16x16 complex unitary U from 12 scalars,
computes a = Re(U @ x) (x real => a = Re(U) @ x), and returns
a.T @ (M @ a) with M = I8 (x) Z = diag(+1,-1,...).  Folding:
    out = x.T @ G @ x,   G = Ur.T @ M @ Ur   (16x16 symmetric, Ur = Re(U))
The G build is O(16x16) scalar work replicated on host; all O(B^2)
work runs on the 8 NeuronCores.

v1: row-sharded full output, fp32 matmuls (accuracy-gold baseline).
v2 (default): output is symmetric, so only the 136 upper-triangle
    512x512 tiles are computed (17 per core — perfectly balanced),
    host mirrors the rest.  Matmuls run as bf16 hi/lo 3-pass
    (hi*hi + hi*lo + lo*hi), giving ~1e-5 relative accuracy at the
    bf16 streaming rate (fp32 matmul is ~4x slower per column).

This walrus build encodes at most ONE sync-wait per instruction, while
the Tile scheduler attaches several; _legalize_waits() hoists extras
onto same-engine NoOps.
"""

import numpy as np

_NCORES = 8
_B = 8192
_SH = _B // _NCORES  # 1024 output rows per core (v1)
_CACHE = {}

# v1 packed input layout: [ G (16) | xr (_SH) | x (_B) ] along the free dim
_OFF_G = 0
_OFF_XR = 16
_OFF_X = 16 + _SH
_W = 16 + _SH + _B

# v2: 16 row-blocks of 512; core i owns row-blocks {i, 15-i} => 17 tiles/core
_NT = 17
_TS = 512
# v2 packed bf16 layout: [Gh(16) | Gl(16) | xrh | xrl | xch | xcl]
_L = _NT * _TS
_OFF2_GH = 0
_OFF2_GL = 16
_OFF2_XRH = 32
_OFF2_XRL = 32 + _L
_OFF2_XCH = 32 + 2 * _L
_OFF2_XCL = 32 + 3 * _L
_W2 = 32 + 4 * _L


def _build_G(phi_1, theta_1, omega_1, phi_2, theta_2, omega_2,
             phi_3, theta_3, omega_3, phi_4, theta_4, omega_4):
    def u(phi, theta, omega):
        phi = float(np.asarray(phi).reshape(-1)[0])
        theta = float(np.asarray(theta).reshape(-1)[0])
        omega = float(np.asarray(omega).reshape(-1)[0])
        half = theta / 2.0
        c, s = np.cos(half), np.sin(half)
        return np.array(
            [[c * np.exp(-1j * (phi + omega) / 2), -s * np.exp(1j * (phi - omega) / 2)],
             [s * np.exp(-1j * (phi - omega) / 2), c * np.exp(1j * (phi + omega) / 2)]],
            dtype=np.complex128)

    CNOT = np.array([[1, 0, 0, 0], [0, 1, 0, 0], [0, 0, 0, 1], [0, 0, 1, 0]],
                    dtype=np.float64)
    I2 = np.eye(2, dtype=np.float64)
    Z = np.array([[1.0, 0.0], [0.0, -1.0]], dtype=np.float64)

    g1 = u(phi_1, theta_1, omega_1)
    # NOTE: gate 2 intentionally uses (phi_2, theta_3, omega_4), as in the source.
    g2 = u(phi_2, theta_3, omega_4)
    g3 = u(phi_3, theta_3, omega_3)
    g4 = u(phi_4, theta_4, omega_4)
    layer_1 = np.kron(np.kron(np.kron(g1, g2), g3), g4)

    layer_2 = np.kron(np.kron(CNOT, I2), I2).astype(np.complex128)
    layer_3 = np.kron(np.kron(I2, CNOT), I2).astype(np.complex128)
    l4_real = np.kron(np.kron(I2, I2), CNOT)
    layer_4 = l4_real.astype(np.complex128)
    layer_5 = (l4_real.reshape((2,) * 8).transpose(0, 2, 1, 3, 4, 6, 5, 7)
               .reshape(16, 16).astype(np.complex128))

    U = layer_5 @ (layer_4 @ (layer_3 @ (layer_2 @ layer_1)))
    M = np.kron(np.kron(np.kron(I2, I2), I2), Z)
    Ur = np.real(U)
    G = Ur.T @ (M @ Ur)
    return np.ascontiguousarray(G, dtype=np.float32)


def _legalize_waits(nc):
    """walrus here encodes at most ONE sync-wait per instruction; hoist the
    extras onto same-engine NoOps placed just before the instruction."""
    import concourse.mybir as mybir

    n = 0
    for fn in nc.m.functions:
        for b in fn.blocks:
            new = []
            for inst in b.instructions:
                si = getattr(inst, "sync_info", None)
                ow = list(si.on_wait) if (si is not None and si.on_wait) else []
                if len(ow) > 1:
                    for w in ow[:-1]:
                        nop = mybir.InstNoOp()
                        nop.engine = inst.engine
                        nop.name = f"legal-nop-{n}"
                        nop.sync_info = mybir.SyncInfo(on_wait=[w], on_update=[])
                        new.append(nop)
                        n += 1
                    si.on_wait = [ow[-1]]
                new.append(inst)
            b.instructions[:] = new
    return nc


# ---------------------------------------------------------------------------
# v1: full output, fp32 matmuls
# ---------------------------------------------------------------------------

def _get_nc():
    if "nc" in _CACHE:
        return _CACHE["nc"]
    import concourse.bass as bass
    import concourse.tile as tile
    import concourse.mybir as mybir

    f32 = mybir.dt.float32
    nc = bass.Bass()
    w_d = nc.dram_tensor("w", [16, _W], f32, kind="ExternalInput")
    out_d = nc.dram_tensor("out", [_SH, _B], f32, kind="ExternalOutput")

    with tile.TileContext(nc) as tc:
        with tc.tile_pool(name="const", bufs=1) as cpool, \
             tc.tile_pool(name="outp", bufs=2) as opool, \
             tc.tile_pool(name="ps_small", bufs=2, space="PSUM") as pss, \
             tc.tile_pool(name="ps_big", bufs=6, space="PSUM") as psb:
            w_sb = cpool.tile([16, _W], f32)
            nc.sync.dma_start(out=w_sb[:], in_=w_d[:])
            g_sb = w_sb[:, _OFF_G:_OFF_G + 16]
            xr_sb = w_sb[:, _OFF_XR:_OFF_XR + _SH]
            x_sb = w_sb[:, _OFF_X:_OFF_X + _B]

            # br = G @ xr  (16, _SH); stationary for the main matmuls.
            br_sb = cpool.tile([16, _SH], f32)
            for j in range(_SH // 512):
                ps = pss.tile([16, 512], f32)
                nc.tensor.matmul(ps[:], g_sb, xr_sb[:, j * 512:(j + 1) * 512],
                                 start=True, stop=True)
                nc.vector.tensor_copy(br_sb[:, j * 512:(j + 1) * 512], ps[:])

            # out rows block m: (128, B) = br[:, m*128:...].T @ x
            for m in range(_SH // 128):
                o_sb = opool.tile([128, _B], f32)
                for n in range(_B // 512):
                    ps = psb.tile([128, 512], f32)
                    nc.tensor.matmul(ps[:], br_sb[:, m * 128:(m + 1) * 128],
                                     x_sb[:, n * 512:(n + 1) * 512],
                                     start=True, stop=True)
                    if n % 2 == 0:
                        nc.vector.tensor_copy(o_sb[:, n * 512:(n + 1) * 512], ps[:])
                    else:
                        nc.scalar.copy(o_sb[:, n * 512:(n + 1) * 512], ps[:])
                nc.sync.dma_start(out=out_d[m * 128:(m + 1) * 128, :], in_=o_sb[:])

    _legalize_waits(nc)
    _CACHE["nc"] = nc
    return nc


def _run(inputs, trace=False, **kw):
    from concourse.bass_utils import run_bass_kernel_spmd

    x = np.ascontiguousarray(np.asarray(inputs["input_state"], dtype=np.float32))
    G = _build_G(**{k: v for k, v in inputs.items() if k != "input_state"})
    nc = _get_nc()
    in_maps = []
    for i in range(_NCORES):
        w = np.empty((16, _W), dtype=np.float32)
        w[:, _OFF_G:_OFF_G + 16] = G
        w[:, _OFF_XR:_OFF_XR + _SH] = x[:, i * _SH:(i + 1) * _SH]
        w[:, _OFF_X:_OFF_X + _B] = x
        in_maps.append({"w": w})
    res = run_bass_kernel_spmd(nc, in_maps, core_ids=list(range(_NCORES)),
                               trace=trace, **kw)
    out = np.concatenate([res.results[i]["out"] for i in range(_NCORES)], axis=0)
    return np.ascontiguousarray(out, dtype=np.float32), res


# ---------------------------------------------------------------------------
# v2: symmetric upper-triangle tiles + bf16 hi/lo 3-pass matmuls
# ---------------------------------------------------------------------------

def _tiles_for_core(i):
    r1, r2 = i, 15 - i
    return ([(r1, c) for c in range(r1, 16)] +
            [(r2, c) for c in range(r2, 16)])


def _get_nc2():
    if "nc2" in _CACHE:
        return _CACHE["nc2"]
    import concourse.bass as bass
    import concourse.tile as tile
    import concourse.mybir as mybir

    f32 = mybir.dt.float32
    bf16 = mybir.dt.bfloat16
    sub = mybir.AluOpType.subtract
    nc = bass.Bass()
    w_d = nc.dram_tensor("w", [16, _W2], bf16, kind="ExternalInput")
    # p-major tile store: out[p, t, m*512+j] = tile_t[m*128+p, j]
    out_d = nc.dram_tensor("out", [128, _NT, 4 * _TS], f32, kind="ExternalOutput")

    with tile.TileContext(nc) as tc:
        with tc.tile_pool(name="const", bufs=1) as cpool, \
             tc.tile_pool(name="bt", bufs=3) as btpool, \
             tc.tile_pool(name="outp", bufs=3) as opool, \
             tc.tile_pool(name="outp1", bufs=1) as opool1, \
             tc.tile_pool(name="ps_small", bufs=2, space="PSUM") as pss, \
             tc.tile_pool(name="ps_big", bufs=6, space="PSUM") as psb:
            w_sb = cpool.tile([16, _W2], bf16)
            nc.sync.dma_start(out=w_sb[:], in_=w_d[:])
            gh = w_sb[:, _OFF2_GH:_OFF2_GH + 16]
            gl = w_sb[:, _OFF2_GL:_OFF2_GL + 16]

            def do_tile(t, o_sb, off):
                xrh = w_sb[:, _OFF2_XRH + t * _TS:_OFF2_XRH + (t + 1) * _TS]
                xrl = w_sb[:, _OFF2_XRL + t * _TS:_OFF2_XRL + (t + 1) * _TS]
                xch = w_sb[:, _OFF2_XCH + t * _TS:_OFF2_XCH + (t + 1) * _TS]
                xcl = w_sb[:, _OFF2_XCL + t * _TS:_OFF2_XCL + (t + 1) * _TS]
                # bt = G @ xrows[t] (f32 in PSUM), 3-pass hi/lo
                ps_s = pss.tile([16, _TS], f32)
                nc.tensor.matmul(ps_s[:], gh, xrh, start=True, stop=False)
                nc.tensor.matmul(ps_s[:], gh, xrl, start=False, stop=False)
                nc.tensor.matmul(ps_s[:], gl, xrh, start=False, stop=True)
                # split bt into bf16 hi/lo on DVE
                bt_h = btpool.tile([16, _TS], bf16)
                bt_l = btpool.tile([16, _TS], bf16)
                nc.vector.tensor_copy(bt_h[:], ps_s[:])
                nc.vector.tensor_tensor(out=bt_l[:], in0=ps_s[:], in1=bt_h[:],
                                        op=sub)
                # tile(r,c) = bt.T @ xcols[t] in 4 psum chunks, 3-pass each
                for m in range(4):
                    ps = psb.tile([128, _TS], f32)
                    bh_m = bt_h[:, m * 128:(m + 1) * 128]
                    bl_m = bt_l[:, m * 128:(m + 1) * 128]
                    nc.tensor.matmul(ps[:], bh_m, xch, start=True, stop=False)
                    nc.tensor.matmul(ps[:], bh_m, xcl, start=False, stop=False)
                    nc.tensor.matmul(ps[:], bl_m, xch, start=False, stop=True)
                    dst = o_sb[:, off + m * _TS:off + (m + 1) * _TS]
                    if (t * 4 + m) % 2 == 0:
                        nc.vector.tensor_copy(dst, ps[:])
                    else:
                        nc.scalar.copy(dst, ps[:])

            for tp in range(_NT // 2):
                o_sb = opool.tile([128, 2 * 4 * _TS], f32)
                do_tile(2 * tp, o_sb, 0)
                do_tile(2 * tp + 1, o_sb, 4 * _TS)
                nc.sync.dma_start(out=out_d[:, 2 * tp:2 * tp + 2, :], in_=o_sb[:])
            o_last = opool1.tile([128, 4 * _TS], f32)
            do_tile(_NT - 1, o_last, 0)
            nc.sync.dma_start(out=out_d[:, _NT - 1, :], in_=o_last[:])

    _legalize_waits(nc)
    _CACHE["nc2"] = nc
    return nc


def _run2(inputs, trace=False, **kw):
    import ml_dtypes
    from concourse.bass_utils import run_bass_kernel_spmd

    bf = ml_dtypes.bfloat16
    x = np.ascontiguousarray(np.asarray(inputs["input_state"], dtype=np.float32))
    G = _build_G(**{k: v for k, v in inputs.items() if k != "input_state"})
    gh = G.astype(bf)
    gl = (G - gh.astype(np.float32)).astype(bf)
    xh = x.astype(bf)
    xl = (x - xh.astype(np.float32)).astype(bf)

    nc = _get_nc2()
    in_maps = []
    for i in range(_NCORES):
        w = np.empty((16, _W2), dtype=bf)
        w[:, _OFF2_GH:_OFF2_GH + 16] = gh
        w[:, _OFF2_GL:_OFF2_GL + 16] = gl
        for t, (r, c) in enumerate(_tiles_for_core(i)):
            rs, cs = slice(r * _TS, (r + 1) * _TS), slice(c * _TS, (c + 1) * _TS)
            w[:, _OFF2_XRH + t * _TS:_OFF2_XRH + (t + 1) * _TS] = xh[:, rs]
            w[:, _OFF2_XRL + t * _TS:_OFF2_XRL + (t + 1) * _TS] = xl[:, rs]
            w[:, _OFF2_XCH + t * _TS:_OFF2_XCH + (t + 1) * _TS] = xh[:, cs]
            w[:, _OFF2_XCL + t * _TS:_OFF2_XCL + (t + 1) * _TS] = xl[:, cs]
        in_maps.append({"w": w})
    res = run_bass_kernel_spmd(nc, in_maps, core_ids=list(range(_NCORES)),
                               trace=trace, **kw)
    out = np.empty((_B, _B), dtype=np.float32)
    for i in range(_NCORES):
        arr = res.results[i]["out"]  # (128, 17, 2048)
        tiles = np.ascontiguousarray(
            arr.reshape(128, _NT, 4, _TS).transpose(1, 2, 0, 3)
        ).reshape(_NT, _TS, _TS)
        for t, (r, c) in enumerate(_tiles_for_core(i)):
            out[r * _TS:(r + 1) * _TS, c * _TS:(c + 1) * _TS] = tiles[t]
            if c > r:
                out[c * _TS:(c + 1) * _TS, r * _TS:(r + 1) * _TS] = tiles[t].T
    return out, res


# ---------------------------------------------------------------------------
# v3: v2 + 16-way PE array packing (32x32 tile mode).
# K=16 uses 16 of 128 PE rows; with tile_position the array splits into
# 4x4 independent 32x32 tiles.  The small matmul runs col-tiled with
# zero-padded G weights, producing bt replicated into all 4 row-group
# partition ranges of one PSUM bank; the 4x3 big matmuls per pass then
# run 16-way concurrent (row group = output sub-block m, col group = c).
# Odd row groups keep operands at +16 within their 32-partition window so
# the four xcols replica DMAs land on disjoint SDMA engine sets.
# ---------------------------------------------------------------------------

_DELTA = (0, 0, 0, 0)  # weights/ifmap must start 32-aligned
# v3 wa layout (bf16): [gh0|gh1|gl0|gl1 (32 each) | xrh (L) | xrl (L)]
_OFF3_XRH = 128
_OFF3_XRL = 128 + _L
_W3A = 128 + 2 * _L
_W3C = 2 * _L  # wc layout: [xch (L) | xcl (L)]


def _get_nc3():
    if "nc3" in _CACHE:
        return _CACHE["nc3"]
    import concourse.bass as bass
    import concourse.tile as tile
    import concourse.mybir as mybir

    f32 = mybir.dt.float32
    bf16 = mybir.dt.bfloat16
    sub = mybir.AluOpType.subtract
    nc = bass.Bass()
    wa_d = nc.dram_tensor("wa", [16, _W3A], bf16, kind="ExternalInput")
    wc_d = nc.dram_tensor("wc", [16, _W3C], bf16, kind="ExternalInput")
    out_d = nc.dram_tensor("out", [128, _NT, 4 * _TS], f32, kind="ExternalOutput")

    with tile.TileContext(nc) as tc:
        with tc.tile_pool(name="const", bufs=1) as cpool, \
             tc.tile_pool(name="bt", bufs=3) as btpool, \
             tc.tile_pool(name="outp", bufs=3) as opool, \
             tc.tile_pool(name="outp1", bufs=1) as opool1, \
             tc.tile_pool(name="ps_small", bufs=2, space="PSUM") as pss, \
             tc.tile_pool(name="ps_big", bufs=6, space="PSUM") as psb:
            wa_sb = cpool.tile([16, _W3A], bf16)
            xc_sb = cpool.tile([128, _W3C], bf16)
            nc.sync.dma_start(out=wa_sb[:], in_=wa_d[:])
            # xcols replicated into each row group's operand window; the four
            # destination partition sets map to disjoint SDMA engine sets.
            for g in range(4):
                base = 32 * g + _DELTA[g]
                nc.scalar.dma_start(out=xc_sb[base:base + 16, :], in_=wc_d[:])
            gW = [wa_sb[:, 32 * v:32 * (v + 1)] for v in range(4)]  # gh0 gh1 gl0 gl1

            def do_tile(t, o_sb, off):
                xrh = wa_sb[:, _OFF3_XRH + t * _TS:_OFF3_XRH + (t + 1) * _TS]
                xrl = wa_sb[:, _OFF3_XRL + t * _TS:_OFF3_XRL + (t + 1) * _TS]
                # small: bt = G @ xrows[t], col-tiled -> 4 replicas in one bank,
                # row-group m's replica sits at partitions 32m+delta..+16
                psq = pss.tile([128, _TS], f32)
                for p, (wsel, rh) in enumerate([(0, xrh), (0, xrl), (2, xrh)]):
                    for m in range(4):
                        lhsT = gW[wsel]
                        nc.tensor.matmul(psq[32 * m:32 * m + 32, :], lhsT, rh,
                                         start=(p == 0), stop=(p == 2),
                                         tile_position=(0, 32 * m))
                bq_h = btpool.tile([128, _TS], bf16)
                bq_l = btpool.tile([128, _TS], bf16)
                nc.vector.tensor_copy(bq_h[:], psq[:])
                nc.vector.tensor_tensor(out=bq_l[:], in0=psq[:], in1=bq_h[:],
                                        op=sub)
                # big: 3 passes x 16-way (row group m x col group c)
                ps_m = [psb.tile([128, _TS], f32, name="psm", tag="psm")
                        for _ in range(4)]
                for p in range(3):
                    bq = bq_h if p < 2 else bq_l
                    xoff = _L if p == 1 else 0
                    for m in range(4):
                        base = 32 * m + _DELTA[m]
                        rhs = xc_sb[base:base + 16,
                                    xoff + t * _TS:xoff + (t + 1) * _TS]
                        for c in range(4):
                            lhsT = bq[base:base + 16,
                                      m * 128 + 32 * c:m * 128 + 32 * (c + 1)]
                            nc.tensor.matmul(
                                ps_m[m][32 * c:32 * c + 32, :], lhsT, rhs,
                                start=(p == 0), stop=(p == 2),
                                tile_position=(32 * m, 32 * c))
                for m in range(4):
                    dst = o_sb[:, off + m * _TS:off + (m + 1) * _TS]
                    if (t * 4 + m) % 2 == 0:
                        nc.vector.tensor_copy(dst, ps_m[m][:])
                    else:
                        nc.scalar.copy(dst, ps_m[m][:])

            for tp in range(_NT // 2):
                o_sb = opool.tile([128, 2 * 4 * _TS], f32)
                do_tile(2 * tp, o_sb, 0)
                do_tile(2 * tp + 1, o_sb, 4 * _TS)
                nc.sync.dma_start(out=out_d[:, 2 * tp:2 * tp + 2, :], in_=o_sb[:])
            o_last = opool1.tile([128, 4 * _TS], f32)
            do_tile(_NT - 1, o_last, 0)
            nc.sync.dma_start(out=out_d[:, _NT - 1, :], in_=o_last[:])

    _legalize_waits(nc)
    _CACHE["nc3"] = nc
    return nc


def _run3(inputs, trace=False, **kw):
    return _run_packed(_get_nc3, inputs, trace=trace, **kw)


def _run_packed(get_nc, inputs, trace=False, **kw):
    import ml_dtypes
    from concourse.bass_utils import run_bass_kernel_spmd

    bf = ml_dtypes.bfloat16
    x = np.ascontiguousarray(np.asarray(inputs["input_state"], dtype=np.float32))
    G = _build_G(**{k: v for k, v in inputs.items() if k != "input_state"})
    gh = G.astype(bf)
    gl = (G - gh.astype(np.float32)).astype(bf)
    xh = x.astype(bf)
    xl = (x - xh.astype(np.float32)).astype(bf)

    nc = get_nc()
    in_maps = []
    for i in range(_NCORES):
        wa = np.zeros((16, _W3A), dtype=bf)
        wa[:, 0:16] = gh          # gh0: data in cols 0-15
        wa[:, 48:64] = gh         # gh1: data in cols 16-31 of slot 1
        wa[:, 64:80] = gl         # gl0
        wa[:, 112:128] = gl       # gl1
        wc = np.empty((16, _W3C), dtype=bf)
        for t, (r, c) in enumerate(_tiles_for_core(i)):
            rs, cs = slice(r * _TS, (r + 1) * _TS), slice(c * _TS, (c + 1) * _TS)
            wa[:, _OFF3_XRH + t * _TS:_OFF3_XRH + (t + 1) * _TS] = xh[:, rs]
            wa[:, _OFF3_XRL + t * _TS:_OFF3_XRL + (t + 1) * _TS] = xl[:, rs]
            wc[:, t * _TS:(t + 1) * _TS] = xh[:, cs]
            wc[:, _L + t * _TS:_L + (t + 1) * _TS] = xl[:, cs]
        in_maps.append({"wa": wa, "wc": wc})
    res = run_bass_kernel_spmd(nc, in_maps, core_ids=list(range(_NCORES)),
                               trace=trace, **kw)
    out = np.empty((_B, _B), dtype=np.float32)
    for i in range(_NCORES):
        arr = res.results[i]["out"]  # (128, 17, 2048)
        tiles = np.ascontiguousarray(
            arr.reshape(128, _NT, 4, _TS).transpose(1, 2, 0, 3)
        ).reshape(_NT, _TS, _TS)
        for t, (r, c) in enumerate(_tiles_for_core(i)):
            out[r * _TS:(r + 1) * _TS, c * _TS:(c + 1) * _TS] = tiles[t]
            if c > r:
                out[c * _TS:(c + 1) * _TS, r * _TS:(r + 1) * _TS] = tiles[t].T
    return out, res


# ---------------------------------------------------------------------------
# v4: v3 + startup fixes — replica loads issued first in fine interleaved
# chunks (they gate the first big matmuls), wa load split so early tiles'
# xrows arrive before the rest, and the psum->bf16 cast moved to ACT to
# balance DVE/ACT evacuation load.
# ---------------------------------------------------------------------------

def _thin_pe_sem(nc):
    """Tile attaches a PE-sem increment to every PE instruction; each inc
    serializes ~26ns at the end of a concurrent tile_position wave.  Keep
    only increments whose cumulative tick some wait actually references,
    and renumber all PE-sem waits accordingly."""
    blocks = [b for fn in nc.m.functions for b in fn.blocks]
    # the PE engine semaphore: updated by PE-engine instructions
    pe_sems = set()
    for b in blocks:
        for i in b.instructions:
            si = getattr(i, "sync_info", None)
            if si and "PE" in str(i.engine):
                for u in (si.on_update or []):
                    if u.ant_name.startswith("PE"):
                        pe_sems.add(u.ant_name)
    if len(pe_sems) != 1:
        return nc  # unexpected; skip optimization
    sem = next(iter(pe_sems))
    needed = set()
    for b in blocks:
        for i in b.instructions:
            si = getattr(i, "sync_info", None)
            if not si:
                continue
            for w in (si.on_wait or []):
                if w.ant_name == sem:
                    assert w.wait_mode == "sem-ge-imm", w.wait_mode
                    needed.add(w.wait_value)
    remap = {}
    old = new = 0
    for b in blocks:
        for i in b.instructions:
            si = getattr(i, "sync_info", None)
            if not si or not si.on_update:
                continue
            ups = list(si.on_update)
            keep = []
            changed = False
            for u in ups:
                if u.ant_name == sem and u.update_mode in ("sem-inc", "sem-add-imm"):
                    assert (u.update_value or 1) == 1, u.update_value
                    old += 1
                    if old in needed:
                        new += 1
                        remap[old] = new
                        keep.append(u)
                    else:
                        changed = True
                else:
                    keep.append(u)
            if changed:
                si.on_update = keep
    seen = set()
    for b in blocks:
        for i in b.instructions:
            si = getattr(i, "sync_info", None)
            if not si or not si.on_wait:
                continue
            ws = list(si.on_wait)
            changed = False
            for w in ws:
                if w.ant_name == sem and id(w) not in seen:
                    seen.add(id(w))
                    w.wait_value = remap[w.wait_value]
                    changed = True
            if changed:
                si.on_wait = ws
    return nc


def _get_nc4(_thin=False, _key="nc4"):
    if _key in _CACHE:
        return _CACHE[_key]
    import concourse.bass as bass
    import concourse.tile as tile
    import concourse.mybir as mybir

    f32 = mybir.dt.float32
    bf16 = mybir.dt.bfloat16
    sub = mybir.AluOpType.subtract
    nc = bass.Bass()
    wa_d = nc.dram_tensor("wa", [16, _W3A], bf16, kind="ExternalInput")
    wc_d = nc.dram_tensor("wc", [16, _W3C], bf16, kind="ExternalInput")
    out_d = nc.dram_tensor("out", [128, _NT, 4 * _TS], f32, kind="ExternalOutput")

    with tile.TileContext(nc) as tc:
        with tc.tile_pool(name="const", bufs=1) as cpool, \
             tc.tile_pool(name="bt", bufs=3) as btpool, \
             tc.tile_pool(name="outp", bufs=3) as opool, \
             tc.tile_pool(name="outp1", bufs=1) as opool1, \
             tc.tile_pool(name="ps_small", bufs=2, space="PSUM") as pss, \
             tc.tile_pool(name="ps_big", bufs=6, space="PSUM") as psb:
            wa_sb = cpool.tile([16, _W3A], bf16)
            xc_sb = cpool.tile([128, _W3C], bf16)
            # xcols replicas gate the big matmuls: issue them FIRST, in
            # column chunks interleaved across the four row groups so the
            # early tiles' columns land before the rest.
            _NCH = 2
            ch = _L // _NCH
            for k in range(_NCH):
                for g in range(4):
                    base = 32 * g
                    for half in range(2):  # xch, xcl
                        o = half * _L + k * ch
                        nc.scalar.dma_start(out=xc_sb[base:base + 16, o:o + ch],
                                            in_=wc_d[:, o:o + ch])
            # wa: G + early xrows first
            hw = 8 * _TS
            nc.sync.dma_start(out=wa_sb[:, 0:128], in_=wa_d[:, 0:128])
            nc.sync.dma_start(out=wa_sb[:, _OFF3_XRH:_OFF3_XRH + hw],
                              in_=wa_d[:, _OFF3_XRH:_OFF3_XRH + hw])
            nc.sync.dma_start(out=wa_sb[:, _OFF3_XRL:_OFF3_XRL + hw],
                              in_=wa_d[:, _OFF3_XRL:_OFF3_XRL + hw])
            nc.sync.dma_start(out=wa_sb[:, _OFF3_XRH + hw:_OFF3_XRH + _L],
                              in_=wa_d[:, _OFF3_XRH + hw:_OFF3_XRH + _L])
            nc.sync.dma_start(out=wa_sb[:, _OFF3_XRL + hw:_OFF3_XRL + _L],
                              in_=wa_d[:, _OFF3_XRL + hw:_OFF3_XRL + _L])
            gW = [wa_sb[:, 32 * v:32 * (v + 1)] for v in range(4)]

            def do_tile(t, o_sb, off):
                xrh = wa_sb[:, _OFF3_XRH + t * _TS:_OFF3_XRH + (t + 1) * _TS]
                xrl = wa_sb[:, _OFF3_XRL + t * _TS:_OFF3_XRL + (t + 1) * _TS]
                psq = pss.tile([128, _TS], f32)
                for p, (wsel, rh) in enumerate([(0, xrh), (0, xrl), (2, xrh)]):
                    for m in range(4):
                        lhsT = gW[wsel]
                        nc.tensor.matmul(psq[32 * m:32 * m + 32, :], lhsT, rh,
                                         start=(p == 0), stop=(p == 2),
                                         tile_position=(0, 32 * m))
                bq_h = btpool.tile([128, _TS], bf16)
                bq_l = btpool.tile([128, _TS], bf16)
                nc.scalar.copy(bq_h[:], psq[:])
                nc.vector.tensor_tensor(out=bq_l[:], in0=psq[:], in1=bq_h[:],
                                        op=sub)
                ps_m = [psb.tile([128, _TS], f32, name="psm", tag="psm")
                        for _ in range(4)]
                for p in range(3):
                    bq = bq_h if p < 2 else bq_l
                    xoff = _L if p == 1 else 0
                    for m in range(4):
                        base = 32 * m
                        rhs = xc_sb[base:base + 16,
                                    xoff + t * _TS:xoff + (t + 1) * _TS]
                        for c in range(4):
                            lhsT = bq[base:base + 16,
                                      m * 128 + 32 * c:m * 128 + 32 * (c + 1)]
                            nc.tensor.matmul(
                                ps_m[m][32 * c:32 * c + 32, :], lhsT, rhs,
                                start=(p == 0), stop=(p == 2),
                                tile_position=(32 * m, 32 * c))
                for m in range(4):
                    dst = o_sb[:, off + m * _TS:off + (m + 1) * _TS]
                    if (t * 4 + m) % 2 == 0:
                        nc.vector.tensor_copy(dst, ps_m[m][:])
                    else:
                        nc.scalar.copy(dst, ps_m[m][:])

            for tp in range(_NT // 2):
                o_sb = opool.tile([128, 2 * 4 * _TS], f32)
                do_tile(2 * tp, o_sb, 0)
                do_tile(2 * tp + 1, o_sb, 4 * _TS)
                nc.sync.dma_start(out=out_d[:, 2 * tp:2 * tp + 2, :], in_=o_sb[:])
            o_last = opool1.tile([128, 4 * _TS], f32)
            do_tile(_NT - 1, o_last, 0)
            nc.sync.dma_start(out=out_d[:, _NT - 1, :], in_=o_last[:])

    if _thin:
        _thin_pe_sem(nc)
    _legalize_waits(nc)
    _CACHE[_key] = nc
    return nc


def _get_nc5():
    return _get_nc4(_thin=True, _key="nc5")


def _run4(inputs, trace=False, **kw):
    return _run_packed(_get_nc4, inputs, trace=trace, **kw)


def _run5(inputs, trace=False, **kw):
    return _run_packed(_get_nc5, inputs, trace=trace, **kw)


# ---------------------------------------------------------------------------
# v6: uniform (32,128) PE tile mode.  v5's 16-way (32,32) waves were SBUF
# read-port bound: the 4 col-tiles of a row group stream the SAME rhs
# partitions concurrently (~1us/wave instead of 0.43).  With M=128 per
# matmul each row group streams its rhs exactly once: smalls become 3
# serial MMs against a col-replicated Gstack weight, bigs 3 waves of 4
# row-group-concurrent MMs.  15 MMs/tile (vs 60), no mode switches.
# ---------------------------------------------------------------------------

# v6 wa layout (bf16): [GstackH (128) | GstackL (128) | xrh (L) | xrl (L)]
_OFF6_XRH = 256
_OFF6_XRL = 256 + _L
_W6A = 256 + 2 * _L


def _get_nc6():
    if "nc6" in _CACHE:
        return _CACHE["nc6"]
    import concourse.bass as bass
    import concourse.tile as tile
    import concourse.mybir as mybir

    f32 = mybir.dt.float32
    bf16 = mybir.dt.bfloat16
    sub = mybir.AluOpType.subtract
    nc = bass.Bass()
    wa_d = nc.dram_tensor("wa", [16, _W6A], bf16, kind="ExternalInput")
    wc_d = nc.dram_tensor("wc", [16, _W3C], bf16, kind="ExternalInput")
    out_d = nc.dram_tensor("out", [128, _NT, 4 * _TS], f32, kind="ExternalOutput")

    with tile.TileContext(nc) as tc:
        with tc.tile_pool(name="const", bufs=1) as cpool, \
             tc.tile_pool(name="bt", bufs=3) as btpool, \
             tc.tile_pool(name="outp", bufs=3) as opool, \
             tc.tile_pool(name="outp1", bufs=1) as opool1, \
             tc.tile_pool(name="ps_small", bufs=2, space="PSUM") as pss, \
             tc.tile_pool(name="ps_big", bufs=6, space="PSUM") as psb:
            wa_sb = cpool.tile([16, _W6A], bf16)
            xc_sb = cpool.tile([128, _W3C], bf16)
            # xcols replicas gate the big matmuls: tile 0-1 columns first,
            # then the rest in two chunks per (group, half).
            bounds = [0, 2 * _TS, 9 * _TS, _L]
            for k in range(len(bounds) - 1):
                a, b = bounds[k], bounds[k + 1]
                for g in range(4):
                    base = 32 * g
                    for half in range(2):
                        o = half * _L
                        nc.scalar.dma_start(
                            out=xc_sb[base:base + 16, o + a:o + b],
                            in_=wc_d[:, o + a:o + b])
            hw = 8 * _TS
            nc.sync.dma_start(out=wa_sb[:, 0:256], in_=wa_d[:, 0:256])
            nc.sync.dma_start(out=wa_sb[:, _OFF6_XRH:_OFF6_XRH + hw],
                              in_=wa_d[:, _OFF6_XRH:_OFF6_XRH + hw])
            nc.sync.dma_start(out=wa_sb[:, _OFF6_XRL:_OFF6_XRL + hw],
                              in_=wa_d[:, _OFF6_XRL:_OFF6_XRL + hw])
            nc.sync.dma_start(out=wa_sb[:, _OFF6_XRH + hw:_OFF6_XRH + _L],
                              in_=wa_d[:, _OFF6_XRH + hw:_OFF6_XRH + _L])
            nc.sync.dma_start(out=wa_sb[:, _OFF6_XRL + hw:_OFF6_XRL + _L],
                              in_=wa_d[:, _OFF6_XRL + hw:_OFF6_XRL + _L])
            gsh = wa_sb[:, 0:128]
            gsl = wa_sb[:, 128:256]

            def do_tile(t, o_sb, off):
                xrh = wa_sb[:, _OFF6_XRH + t * _TS:_OFF6_XRH + (t + 1) * _TS]
                xrl = wa_sb[:, _OFF6_XRL + t * _TS:_OFF6_XRL + (t + 1) * _TS]
                # bt replicated into the 4 row-group windows of one bank via
                # the col-replicated Gstack weight; one MM per pass.
                psq = pss.tile([128, _TS], f32)
                nc.tensor.matmul(psq[:], gsh, xrh, start=True, stop=False,
                                 tile_position=(0, 0))
                nc.tensor.matmul(psq[:], gsh, xrl, start=False, stop=False,
                                 tile_position=(0, 0))
                nc.tensor.matmul(psq[:], gsl, xrh, start=False, stop=True,
                                 tile_position=(0, 0))
                bq_h = btpool.tile([128, _TS], bf16)
                bq_l = btpool.tile([128, _TS], bf16)
                nc.scalar.copy(bq_h[:], psq[:])
                nc.vector.tensor_tensor(out=bq_l[:], in0=psq[:], in1=bq_h[:],
                                        op=sub)
                ps_m = [psb.tile([128, _TS], f32, name="psm", tag="psm")
                        for _ in range(4)]
                for p in range(3):
                    bq = bq_h if p < 2 else bq_l
                    xoff = _L if p == 1 else 0
                    for m in range(4):
                        base = 32 * m
                        rhs = xc_sb[base:base + 16,
                                    xoff + t * _TS:xoff + (t + 1) * _TS]
                        lhsT = bq[base:base + 16, m * 128:(m + 1) * 128]
                        nc.tensor.matmul(ps_m[m][:], lhsT, rhs,
                                         start=(p == 0), stop=(p == 2),
                                         tile_position=(32 * m, 0))
                for m in range(4):
                    dst = o_sb[:, off + m * _TS:off + (m + 1) * _TS]
                    if (t * 4 + m) % 2 == 0:
                        nc.vector.tensor_copy(dst, ps_m[m][:])
                    else:
                        nc.scalar.copy(dst, ps_m[m][:])

            for tp in range(_NT // 2):
                o_sb = opool.tile([128, 2 * 4 * _TS], f32)
                do_tile(2 * tp, o_sb, 0)
                do_tile(2 * tp + 1, o_sb, 4 * _TS)
                nc.sync.dma_start(out=out_d[:, 2 * tp:2 * tp + 2, :], in_=o_sb[:])
            o_last = opool1.tile([128, 4 * _TS], f32)
            do_tile(_NT - 1, o_last, 0)
            nc.sync.dma_start(out=out_d[:, _NT - 1, :], in_=o_last[:])

    _thin_pe_sem(nc)
    _legalize_waits(nc)
    _CACHE["nc6"] = nc
    return nc


def _run6(inputs, trace=False, **kw):
    return _run_packed6(_get_nc6, inputs, trace=trace, **kw)


def _run_packed6(get_nc, inputs, trace=False, **kw):
    import ml_dtypes
    from concourse.bass_utils import run_bass_kernel_spmd

    bf = ml_dtypes.bfloat16
    x = np.ascontiguousarray(np.asarray(inputs["input_state"], dtype=np.float32))
    G = _build_G(**{k: v for k, v in inputs.items() if k != "input_state"})
    gh = G.astype(bf)
    gl = (G - gh.astype(np.float32)).astype(bf)
    xh = x.astype(bf)
    xl = (x - xh.astype(np.float32)).astype(bf)
    gstack_h = np.zeros((16, 128), dtype=bf)
    gstack_l = np.zeros((16, 128), dtype=bf)
    for c in range(4):
        gstack_h[:, 32 * c:32 * c + 16] = gh
        gstack_l[:, 32 * c:32 * c + 16] = gl

    nc = get_nc()
    in_maps = []
    for i in range(_NCORES):
        wa = np.zeros((16, _W6A), dtype=bf)
        wa[:, 0:128] = gstack_h
        wa[:, 128:256] = gstack_l
        wc = np.empty((16, _W3C), dtype=bf)
        for t, (r, c) in enumerate(_tiles_for_core(i)):
            rs, cs = slice(r * _TS, (r + 1) * _TS), slice(c * _TS, (c + 1) * _TS)
            wa[:, _OFF6_XRH + t * _TS:_OFF6_XRH + (t + 1) * _TS] = xh[:, rs]
            wa[:, _OFF6_XRL + t * _TS:_OFF6_XRL + (t + 1) * _TS] = xl[:, rs]
            wc[:, t * _TS:(t + 1) * _TS] = xh[:, cs]
            wc[:, _L + t * _TS:_L + (t + 1) * _TS] = xl[:, cs]
        in_maps.append({"wa": wa, "wc": wc})
    res = run_bass_kernel_spmd(nc, in_maps, core_ids=list(range(_NCORES)),
                               trace=trace, **kw)
    out = np.empty((_B, _B), dtype=np.float32)
    for i in range(_NCORES):
        arr = res.results[i]["out"]
        tiles = np.ascontiguousarray(
            arr.reshape(128, _NT, 4, _TS).transpose(1, 2, 0, 3)
        ).reshape(_NT, _TS, _TS)
        for t, (r, c) in enumerate(_tiles_for_core(i)):
            out[r * _TS:(r + 1) * _TS, c * _TS:(c + 1) * _TS] = tiles[t]
            if c > r:
                out[c * _TS:(c + 1) * _TS, r * _TS:(r + 1) * _TS] = tiles[t].T
    return out, res


# ---------------------------------------------------------------------------
# v7: v6 + replica loads split across BOTH HWDGE rings (row groups 0/1 on
# the scalar ring hit SBUF ports {0,2,4,6}, groups 2/3 on the sync ring hit
# {1,3,5,7} — disjoint, so the two rings truly overlap), and per-tile 1MB
# out-DMAs so the kernel tail drains one tile, not a pair.
# ---------------------------------------------------------------------------

def _get_nc7():
    if "nc7" in _CACHE:
        return _CACHE["nc7"]
    import concourse.bass as bass
    import concourse.tile as tile
    import concourse.mybir as mybir

    f32 = mybir.dt.float32
    bf16 = mybir.dt.bfloat16
    sub = mybir.AluOpType.subtract
    nc = bass.Bass()
    wa_d = nc.dram_tensor("wa", [16, _W6A], bf16, kind="ExternalInput")
    wc_d = nc.dram_tensor("wc", [16, _W3C], bf16, kind="ExternalInput")
    out_d = nc.dram_tensor("out", [128, _NT, 4 * _TS], f32, kind="ExternalOutput")

    with tile.TileContext(nc) as tc:
        with tc.tile_pool(name="const", bufs=1) as cpool, \
             tc.tile_pool(name="bt", bufs=3) as btpool, \
             tc.tile_pool(name="outp", bufs=4) as opool, \
             tc.tile_pool(name="ps_small", bufs=2, space="PSUM") as pss, \
             tc.tile_pool(name="ps_big", bufs=6, space="PSUM") as psb:
            wa_sb = cpool.tile([16, _W6A], bf16)
            xc_sb = cpool.tile([128, _W3C], bf16)
            bounds = [0, 2 * _TS, 9 * _TS, _L]
            hw = 8 * _TS
            # sync ring: G + early xrows, then row-group 2/3 replicas
            nc.sync.dma_start(out=wa_sb[:, 0:256], in_=wa_d[:, 0:256])
            nc.sync.dma_start(out=wa_sb[:, _OFF6_XRH:_OFF6_XRH + hw],
                              in_=wa_d[:, _OFF6_XRH:_OFF6_XRH + hw])
            nc.sync.dma_start(out=wa_sb[:, _OFF6_XRL:_OFF6_XRL + hw],
                              in_=wa_d[:, _OFF6_XRL:_OFF6_XRL + hw])
            for k in range(len(bounds) - 1):
                a, b = bounds[k], bounds[k + 1]
                for g in (2, 3):
                    base = 32 * g
                    for half in range(2):
                        o = half * _L
                        nc.sync.dma_start(
                            out=xc_sb[base:base + 16, o + a:o + b],
                            in_=wc_d[:, o + a:o + b])
                if k == 0:
                    nc.sync.dma_start(
                        out=wa_sb[:, _OFF6_XRH + hw:_OFF6_XRH + _L],
                        in_=wa_d[:, _OFF6_XRH + hw:_OFF6_XRH + _L])
                    nc.sync.dma_start(
                        out=wa_sb[:, _OFF6_XRL + hw:_OFF6_XRL + _L],
                        in_=wa_d[:, _OFF6_XRL + hw:_OFF6_XRL + _L])
            # scalar ring: row-group 0/1 replicas
            for k in range(len(bounds) - 1):
                a, b = bounds[k], bounds[k + 1]
                for g in (0, 1):
                    base = 32 * g
                    for half in range(2):
                        o = half * _L
                        nc.scalar.dma_start(
                            out=xc_sb[base:base + 16, o + a:o + b],
                            in_=wc_d[:, o + a:o + b])
            gsh = wa_sb[:, 0:128]
            gsl = wa_sb[:, 128:256]

            def do_tile(t, o_sb, off):
                xrh = wa_sb[:, _OFF6_XRH + t * _TS:_OFF6_XRH + (t + 1) * _TS]
                xrl = wa_sb[:, _OFF6_XRL + t * _TS:_OFF6_XRL + (t + 1) * _TS]
                psq = pss.tile([128, _TS], f32)
                nc.tensor.matmul(psq[:], gsh, xrh, start=True, stop=False,
                                 tile_position=(0, 0))
                nc.tensor.matmul(psq[:], gsh, xrl, start=False, stop=False,
                                 tile_position=(0, 0))
                nc.tensor.matmul(psq[:], gsl, xrh, start=False, stop=True,
                                 tile_position=(0, 0))
                bq_h = btpool.tile([128, _TS], bf16)
                bq_l = btpool.tile([128, _TS], bf16)
                nc.scalar.copy(bq_h[:], psq[:])
                nc.vector.tensor_tensor(out=bq_l[:], in0=psq[:], in1=bq_h[:],
                                        op=sub)
                ps_m = [psb.tile([128, _TS], f32, name="psm", tag="psm")
                        for _ in range(4)]
                for p in range(3):
                    bq = bq_h if p < 2 else bq_l
                    xoff = _L if p == 1 else 0
                    for m in range(4):
                        base = 32 * m
                        rhs = xc_sb[base:base + 16,
                                    xoff + t * _TS:xoff + (t + 1) * _TS]
                        lhsT = bq[base:base + 16, m * 128:(m + 1) * 128]
                        nc.tensor.matmul(ps_m[m][:], lhsT, rhs,
                                         start=(p == 0), stop=(p == 2),
                                         tile_position=(32 * m, 0))
                for m in range(4):
                    dst = o_sb[:, off + m * _TS:off + (m + 1) * _TS]
                    if (t * 4 + m) % 2 == 0:
                        nc.vector.tensor_copy(dst, ps_m[m][:])
                    else:
                        nc.scalar.copy(dst, ps_m[m][:])

            for t in range(_NT):
                o_sb = opool.tile([128, 4 * _TS], f32)
                do_tile(t, o_sb, 0)
                nc.sync.dma_start(out=out_d[:, t, :], in_=o_sb[:])

    _thin_pe_sem(nc)
    _legalize_waits(nc)
    _CACHE["nc7"] = nc
    return nc


def _run7(inputs, trace=False, **kw):
    return _run_packed6(_get_nc7, inputs, trace=trace, **kw)


_VARIANTS = {"v1": _run, "v2": _run2, "v3": _run3, "v4": _run4,
             "v5": _run5, "v6": _run6, "v7": _run7}


def kernel(**inputs):
    import os
    run = _VARIANTS[os.environ.get("BWN_VARIANT", "v2")]
    out, _ = run(inputs)
    return out
